# revision 23
# baseline (speedup 1.0000x reference)
"""Multi-head forgetting attention on 8 trn2 cores.

Sharding: 4 heads per core as 2 partition-slices ("hp") of 2 heads each
(head/tensor parallel). Each core receives the full (host-pre-transposed)
activations, its column slice of Wq/Wk/Wv, its row slice of Wo^T, and
produces a partial (S,D) bf16 output (both hp accumulated on-chip) which
the host sums in f32 (+ bo).

Gate structure: l = gq(q)+gk(k)+gb is materialised per 128x512 tile by a
2-contraction PE matmul from row-layout gq / gk vectors (bf16, PSUM), so
the ACT tanh pass needs no per-head bias and handles both heads per
instruction: sigma*s = (tanh(l/2)+1) * (s/2), with the 1/2 folded into
the host-side q scaling. For the aq=3 chunks the gate instead goes
through an exact reciprocal path (den = 0.5+0.5*u(q)v(k) on Pool from
exp vectors, reciprocal_approx_fast + multiply on DVE) which moves work
off the ACT engine. exp stays on ACT; the elementwise multiply reads
scores straight from PSUM (scores matmul emits bf16).

Other structure: flash-style streaming over k-tiles per 512-wide q chunk,
mixed 128x128 mask blocks multiplied by deduped 0/1 tiles (loaded once),
x tiles DMAed in 1MB batches, output projection accumulates both hp in
PSUM, one output DMA per 128-row slice.
"""

import os
import sys

sys.path.insert(0, "/opt/trn_rl_repo")

import numpy as np
import ml_dtypes

bf16 = ml_dtypes.bfloat16

B, S, D, H = 2, 2048, 1024, 16
DK = 64
NCORES = 8
HPC = 2          # head-pairs per core
CW = HPC * DK    # 128 per-slice head width
P = 128
QTW = 512        # q tile width (matmul free dim)
NQT = S // QTW   # 4
NKT = S // P     # 16 k tiles
NSL = S // P     # 16 q slices
ND = D // P      # 8 contraction tiles
DKP = DK + 2     # gq/gk row-layout tiles (2 rows per head-pair)
LN2 = 0.6931471805599453

# (aq, hp) chunks routed through the reciprocal gate path (off-ACT)
RECIP_PAIRS = {(3, 0), (3, 1)}

_CACHE = {}


def _prep_mask(mask):
    """Batch-union block table: 0 skip / 1 full / 2 mixed, plus per-batch
    0/1 tile contents for each union-mixed block (cores take their b's).
    Mixed tiles are deduped (consistently across batches) so each unique
    tile is loaded once."""
    m = np.asarray(mask).astype(bool)
    st = np.zeros((NKT, NSL), dtype=np.int8)
    uniq = {}
    m01 = [[] for _ in range(B)]
    midx = {}
    for i in range(NKT):
        for s in range(NSL):
            blks = [m[b, s * P:(s + 1) * P, i * P:(i + 1) * P]
                    for b in range(B)]
            alls = [blk.all() for blk in blks]
            anys = [blk.any() for blk in blks]
            if all(alls):
                st[i, s] = 1
            elif not any(anys):
                st[i, s] = 0
            else:
                st[i, s] = 2
                # used as matmul lhsT (applied transposed): keep blk
                # orientation so sc[k,q] += (blk[q,k]-1)*3e4
                tls = [np.ascontiguousarray(
                        (blks[b].astype(np.float32) - 1.0)
                        * 30000.0).astype(bf16) for b in range(B)]
                key = tuple(t.tobytes() for t in tls)
                if key not in uniq:
                    uniq[key] = len(m01[0])
                    for b in range(B):
                        m01[b].append(tls[b])
                midx[(i, s)] = uniq[key]
    if not m01[0]:
        for b in range(B):
            m01[b].append(np.zeros((P, P), dtype=bf16))
    # SBUF layout [P, n_uniq, P]
    return st, [np.stack(x, axis=1) for x in m01], midx


def _build(meta):
    """Build the (shared-across-cores) bass program."""
    import concourse.mybir as mybir
    import concourse.tile as tile
    from concourse import bacc

    st = meta["st"]
    midx = meta["midx"]
    n_m01 = meta["n_m01"]
    gb = meta["gb"]
    use_bq = meta["use_bq"]
    use_bk = meta["use_bk"]
    use_bv = meta["use_bv"]

    f32 = mybir.dt.float32
    b16 = mybir.dt.bfloat16
    Act = mybir.ActivationFunctionType
    Alu = mybir.AluOpType

    # block tables (batch-union)
    iv_qt = {qt: [i for i in range(NKT)
                  if any(st[i, 4 * qt + j] for j in range(4))]
             for qt in range(NQT)}
    valid_i = {s: [i for i in range(NKT) if st[i, s]]
               for s in range(NSL)}
    # attention chunk qt can only be emitted after projection chunk
    # mc[qt] (the latest chunk producing a k-tile it reads)
    mc = {qt: max((i // (QTW // P) for i in iv_qt[qt]), default=0)
          for qt in range(NQT)}

    nc = bacc.Bacc("TRN2", debug=False, enable_asserts=False,
                   num_devices=NCORES)

    xqt = nc.dram_tensor("xqt", (P, ND, S), b16, kind="ExternalInput")
    xkt = nc.dram_tensor("xkt", (P, ND, S), b16, kind="ExternalInput")
    xvt = nc.dram_tensor("xvt", (P, ND, S), b16, kind="ExternalInput")
    wqt = nc.dram_tensor("wqt", (P, HPC, D), b16, kind="ExternalInput")
    wkt = nc.dram_tensor("wkt", (P, HPC, D), b16, kind="ExternalInput")
    wvt = nc.dram_tensor("wvt", (P, HPC, D), b16, kind="ExternalInput")
    wot = nc.dram_tensor("wot", (P, HPC, D), b16, kind="ExternalInput")
    wgq = nc.dram_tensor("wgq", (P, DKP), b16, kind="ExternalInput")
    wgk = nc.dram_tensor("wgk", (P, HPC), b16, kind="ExternalInput")
    identd = nc.dram_tensor("identd", (P, P), b16, kind="ExternalInput")
    onesd = nc.dram_tensor("onesd", (DK + 1, P), b16, kind="ExternalInput")
    m01d = nc.dram_tensor("m01d", (P, n_m01, P), b16, kind="ExternalInput")
    ebiasd = nc.dram_tensor("ebiasd", (P, 1), f32, kind="ExternalInput")
    gb05d = nc.dram_tensor("gb05d", (P, 1), f32, kind="ExternalInput")
    bqt = nc.dram_tensor("bqt", (P, HPC, 1), f32, kind="ExternalInput")
    bkt = nc.dram_tensor("bkt", (P, HPC, 1), f32, kind="ExternalInput")
    bvt = nc.dram_tensor("bvt", (P, HPC, P), f32, kind="ExternalInput")
    outp = nc.dram_tensor("outp", (S, D), b16, kind="ExternalOutput")

    with tile.TileContext(nc) as tc:
        from contextlib import ExitStack
        with ExitStack() as ctx:
            cst = ctx.enter_context(tc.tile_pool(name="cst", bufs=1))
            per = ctx.enter_context(tc.tile_pool(name="per", bufs=1))
            strm = ctx.enter_context(tc.tile_pool(name="strm", bufs=1))
            work = ctx.enter_context(tc.tile_pool(name="work", bufs=2))
            prb = ctx.enter_context(tc.tile_pool(name="prb", bufs=1))
            mis = ctx.enter_context(
                tc.tile_pool(name="mis", bufs=2, space="PSUM"))
            scp = ctx.enter_context(
                tc.tile_pool(name="scp", bufs=5, space="PSUM"))
            att = ctx.enter_context(
                tc.tile_pool(name="att", bufs=1, space="PSUM"))

            # ---- constants ----
            wq_sb = cst.tile([P, HPC, D], b16, name="wq_sb")
            wk_sb = cst.tile([P, HPC, D], b16, name="wk_sb")
            wv_sb = cst.tile([P, HPC, D], b16, name="wv_sb")
            wo_sb = cst.tile([P, HPC, D], b16, name="wo_sb")
            wgq_sb = cst.tile([P, DKP], b16, name="wgq_sb")
            wgk_sb = cst.tile([P, HPC], b16, name="wgk_sb")
            id_sb = cst.tile([P, P], b16, name="id_sb")
            ones_sb = cst.tile([DK + 1, P], b16, name="ones_sb")
            m01_sb = cst.tile([P, n_m01, P], b16, name="m01_sb")
            ebias = cst.tile([P, 1], f32, name="ebias")
            gb05 = cst.tile([P, 1], f32, name="gb05")
            bq_sb = cst.tile([P, HPC, 1], f32, name="bq_sb")
            bk_sb = cst.tile([P, HPC, 1], f32, name="bk_sb")
            bv_sb = cst.tile([P, HPC, P], f32, name="bv_sb")
            nc.sync.dma_start(wq_sb[:], wqt[:, :, :])
            nc.sync.dma_start(wk_sb[:], wkt[:, :, :])
            nc.sync.dma_start(wv_sb[:], wvt[:, :, :])
            nc.sync.dma_start(wo_sb[:], wot[:, :, :])
            nc.sync.dma_start(wgq_sb[:], wgq[:, :])
            nc.sync.dma_start(wgk_sb[:], wgk[:, :])
            nc.sync.dma_start(id_sb[:], identd[:, :])
            nc.sync.dma_start(ones_sb[:], onesd[:, :])
            nc.scalar.dma_start(m01_sb[:], m01d[:, :, :])
            nc.sync.dma_start(ebias[:], ebiasd[:, :])
            nc.sync.dma_start(gb05[:], gb05d[:, :])
            if use_bq:
                nc.sync.dma_start(bq_sb[:], bqt[:, :, :])
            if use_bk:
                nc.sync.dma_start(bk_sb[:], bkt[:, :, :])
            if use_bv:
                nc.sync.dma_start(bv_sb[:], bvt[:, :, :])

            # per-hp persistent buffers
            qt_sb = [per.tile([P, S], b16, name=f"qt{hp}_sb", tag=f"qt{hp}")
                     for hp in range(HPC)]
            kt_sb = [per.tile([P, S], b16, name=f"kt{hp}_sb", tag=f"kt{hp}")
                     for hp in range(HPC)]
            v2_sb = [per.tile([P, NKT, HPC, DK + 1], b16,
                              name=f"v{hp}_sb", tag=f"v{hp}")
                     for hp in range(HPC)]
            ug2_sb = [per.tile([DKP, S], b16, name=f"ug{hp}_sb",
                               tag=f"ug{hp}") for hp in range(HPC)]
            gqb_sb = [per.tile([P, HPC, S], b16, name=f"gqb{hp}_sb",
                               tag=f"gqb{hp}") for hp in range(HPC)]
            ubc_sb = [per.tile([P, HPC, S], b16, name=f"ubc{hp}_sb",
                               tag=f"ubc{hp}") for hp in range(HPC)]
            gk05_sb = [per.tile([P, HPC, NKT], f32, name=f"gk05{hp}_sb",
                                tag=f"gk05{hp}") for hp in range(HPC)]
            vex_sb = [per.tile([P, HPC, NKT], f32, name=f"vex{hp}_sb",
                               tag=f"vex{hp}") for hp in range(HPC)]

            for hp in range(HPC):
                nc.vector.memset(v2_sb[hp][:, :, :, DK], 1.0)

            for qt in range(NQT):
                # ===== projection chunk qt (x tiles shared by both hp) ====
                q0 = qt * QTW
                # Q / K chunk qt: one batched x tile feeds both hp matmuls
                for (xsrc, wsb, osb, bias_sb, use_b, dmae, xtag) in (
                        (xqt, wq_sb, qt_sb, bq_sb, use_bq, nc.scalar, "xq"),
                        (xkt, wk_sb, kt_sb, bk_sb, use_bk, nc.sync, "xk")):
                    xt = strm.tile([P, ND, QTW], b16, tag=xtag, name="xt")
                    dmae.dma_start(xt[:], xsrc[:, :, q0:q0 + QTW])
                    pps = [scp.tile([P, QTW], f32, tag="sc",
                                    name=f"pps{hp}") for hp in range(HPC)]
                    for dt in range(ND):
                        for hp in range(HPC):
                            nc.tensor.matmul(
                                pps[hp][:],
                                lhsT=wsb[:, hp, dt * P:(dt + 1) * P],
                                rhs=xt[:, dt, :],
                                start=(dt == 0), stop=(dt == ND - 1))
                    for hp in range(HPC):
                        dst = osb[hp][:, q0:q0 + QTW]
                        if use_b:
                            nc.scalar.activation(
                                dst, pps[hp][:], Act.Identity,
                                bias=bias_sb[:, hp, :])
                        elif hp:
                            nc.scalar.copy(dst, pps[hp][:])
                        else:
                            nc.vector.tensor_copy(dst, pps[hp][:])

                # V slices 4qt..4qt+3
                xv = strm.tile([P, ND, QTW], b16, tag="xv", name="xv")
                nc.sync.dma_start(xv[:], xvt[:, :, q0:q0 + QTW])
                for hp in range(HPC):
                    for sj in range(QTW // P):
                        sl = qt * (QTW // P) + sj
                        vps = scp.tile([P, HPC, DK], f32, tag="sc",
                                       name="vps")
                        for dt in range(ND):
                            nc.tensor.matmul(
                                vps[:],
                                lhsT=xv[:, dt, sj * P:(sj + 1) * P],
                                rhs=wv_sb[:, hp, dt * P:(dt + 1) * P],
                                start=(dt == 0), stop=(dt == ND - 1))
                        if use_bv:
                            for h in range(HPC):
                                nc.vector.tensor_add(
                                    vps[:, h, :], vps[:, h, :],
                                    bv_sb[:, hp, h * DK:(h + 1) * DK])
                        # both heads in one strided copy
                        if sj % 2:
                            nc.scalar.copy(
                                v2_sb[hp][:, sl, :, 0:DK], vps[:])
                        else:
                            nc.vector.tensor_copy(
                                v2_sb[hp][:, sl, :, 0:DK], vps[:])

                for hp in range(HPC):
                    # gq row chunk, then its partition-broadcast tiles
                    gps = mis.tile([DKP, QTW], f32, tag="mis", name="gps")
                    nc.tensor.matmul(
                        gps[:], lhsT=wgq_sb[:],
                        rhs=qt_sb[hp][:, q0:q0 + QTW],
                        start=True, stop=True)
                    nc.scalar.copy(ug2_sb[hp][:, q0:q0 + QTW], gps[:])
                    for h in range(HPC):
                        gqp = mis.tile([P, QTW], f32, tag="mis",
                                       name="gqp")
                        nc.tensor.matmul(
                            gqp[:],
                            lhsT=ones_sb[h * DK:h * DK + 1, :],
                            rhs=ug2_sb[hp][h * DK:h * DK + 1,
                                           q0:q0 + QTW],
                            start=True, stop=True)
                        if h:
                            nc.scalar.copy(
                                gqb_sb[hp][:, h, q0:q0 + QTW], gqp[:])
                        else:
                            nc.vector.tensor_copy(
                                gqb_sb[hp][:, h, q0:q0 + QTW], gqp[:])
                        nc.scalar.activation(
                            ubc_sb[hp][:, h, q0:q0 + QTW], gqp[:],
                            Act.Exp, scale=-1.0, bias=ebias[:, :])

                    # per-partition gk: gk05 (tanh bias) + vex (recip)
                    for h in range(HPC):
                        hsl = slice(h * DK, (h + 1) * DK)
                        gkp = mis.tile([P, QTW // P], f32, tag="mis",
                                       name="gkp")
                        for j in range(QTW // P):
                            i = qt * (QTW // P) + j
                            nc.tensor.matmul(
                                gkp[:, j:j + 1],
                                lhsT=kt_sb[hp][hsl, i * P:(i + 1) * P],
                                rhs=wgk_sb[hsl, h:h + 1],
                                start=(j == 0), stop=(j == QTW // P - 1),
                                skip_group_check=True)
                        nc.scalar.activation(
                            gk05_sb[hp][:, h, qt * (QTW // P):
                                        (qt + 1) * (QTW // P)],
                            gkp[:], Act.Identity, scale=0.5,
                            bias=gb05[:, :])
                        nc.scalar.activation(
                            vex_sb[hp][:, h, qt * (QTW // P):
                                       (qt + 1) * (QTW // P)],
                            gkp[:], Act.Exp, scale=-1.0)

                # ===== attention chunks whose k-tiles are now ready ====
                ready = [aq for aq in range(NQT)
                         if mc[aq] == qt or (qt == NQT - 1 and mc[aq] > qt)]
                for aq in ready:
                  a0 = aq * QTW
                  otts = {}
                  for hp in range(HPC):
                      probs = {}
                      tiles = []
                      for n_i, i in enumerate(iv_qt[aq]):
                          recip = (n_i % 3 == 1)
                          # first non-skip 128-slice of this (i, aq)
                          sjlo = min(j for j in range(QTW // P)
                                     if st[i, aq * (QTW // P) + j])
                          tiles.append((i, recip, sjlo * P))
                      # pre-emit gate precursors (tanh on ACT, den on
                      # Pool): all inputs are chunk-level, so these run
                      # ahead of the score pipeline without stalls
                      pre = {}
                      for (i, recip, off) in tiles:
                          for h in range(HPC):
                              if recip:
                                  dn = work.tile([P, QTW], f32,
                                                 tag="den", name="den",
                                                 bufs=8)
                                  nc.gpsimd.tensor_scalar(
                                      dn[:, off:],
                                      ubc_sb[hp][:, h, a0 + off:a0 + QTW],
                                      vex_sb[hp][:, h, i:i + 1], 0.5,
                                      Alu.mult, Alu.add)
                                  pre[(i, h)] = dn
                              else:
                                  tnh = work.tile([P, QTW], b16,
                                                  tag="tnh", name="tnh",
                                                  bufs=24)
                                  nc.scalar.activation(
                                      tnh[:, off:],
                                      gqb_sb[hp][:, h, a0 + off:a0 + QTW],
                                      Act.Tanh, scale=0.5,
                                      bias=gk05_sb[hp][:, h, i:i + 1])
                                  pre[(i, h)] = tnh
                      for (i, recip, off) in tiles:
                          p3 = prb.tile([P, HPC, QTW], b16, tag=f"pr{i}",
                                        name=f"pr{i}",
                                        bufs=2 if i < 4 else 1)
                          g3 = work.tile([P, HPC, QTW], b16, tag="gat",
                                         name="gat")
                          sjlo = off // P
                          mixed = [sj for sj in range(sjlo, QTW // P)
                                   if st[i, aq * (QTW // P) + sj] == 2]
                          for h in range(HPC):
                              hsl = slice(h * DK, (h + 1) * DK)
                              sch = scp.tile([P, QTW], f32, tag="sc",
                                             name="sch")
                              nc.tensor.matmul(
                                  sch[:, off:],
                                  lhsT=kt_sb[hp][hsl, i * P:(i + 1) * P],
                                  rhs=qt_sb[hp][hsl, a0 + off:a0 + QTW],
                                  start=True, stop=not mixed)
                              # additive mask bias: sc += m01^T . I
                              for n, sj in enumerate(mixed):
                                  s = aq * (QTW // P) + sj
                                  nc.tensor.matmul(
                                      sch[:, sj * P:(sj + 1) * P],
                                      lhsT=m01_sb[:, midx[(i, s)], :],
                                      rhs=id_sb[:],
                                      start=False,
                                      stop=(n == len(mixed) - 1),
                                      skip_group_check=True)
                              if recip:
                                  rc = work.tile([P, QTW], f32, tag="rec",
                                                 name="rec", bufs=3)
                                  nc.vector.reciprocal_approx_fast(
                                      rc[:, off:], pre[(i, h)][:, off:])
                                  nc.vector.tensor_tensor(
                                      g3[:, h, off:], sch[:, off:],
                                      rc[:, off:], Alu.mult)
                              else:
                                  nc.vector.scalar_tensor_tensor(
                                      g3[:, h, off:], pre[(i, h)][:, off:],
                                      1.0, sch[:, off:],
                                      Alu.add, Alu.mult)
                          nc.scalar.activation(
                              p3[:, :, off:], g3[:, :, off:], Act.Exp)
                          probs[i] = p3

                      # attn @ V, normalize, transpose; out-proj deferred
                      ott = work.tile([P, QTW // P, P], b16,
                                      tag=f"ott{hp}", name=f"ott{hp}")
                      otts[hp] = ott
                      for sj in range(QTW // P):
                          s = aq * (QTW // P) + sj
                          onat = work.tile([P, P], b16, tag="onat",
                                           name="onat")
                          ops = att.tile([P, HPC * (DK + 1)], f32, tag="o",
                                         name="ops")
                          vi = valid_i[s]
                          for h in range(HPC):
                              ob = h * (DK + 1)
                              if not vi:
                                  nc.vector.memset(
                                      ops[:, ob:ob + DK + 1], 0.0)
                              for n, i in enumerate(vi):
                                  nc.tensor.matmul(
                                      ops[:, ob:ob + DK + 1],
                                      lhsT=probs[i][:, h,
                                                    sj * P:(sj + 1) * P],
                                      rhs=v2_sb[hp][:, i, h, :],
                                      start=(n == 0),
                                      stop=(n == len(vi) - 1),
                                      skip_group_check=True)
                              recv = work.tile([P, 1], f32, tag="recip",
                                               name="recip", bufs=4)
                              nc.vector.reciprocal_approx_fast(
                                  recv[:], ops[:, ob + DK:ob + DK + 1])
                              nc.vector.tensor_scalar_mul(
                                  onat[:, h * DK:(h + 1) * DK],
                                  ops[:, ob:ob + DK], recv[:])
                          trp = mis.tile([P, P], b16, tag="mis", name="trp")
                          nc.tensor.transpose(trp[:], onat[:], id_sb[:])
                          nc.vector.tensor_copy(ott[:, sj, :], trp[:])
                  # output projection: accumulate both hp per s-slice
                  for sj in range(QTW // P):
                      s = aq * (QTW // P) + sj
                      po = work.tile([P, 2, QTW], b16, tag="po",
                                     name="po", bufs=3)
                      for nt in range(2):
                          pps2 = mis.tile([P, QTW], f32, tag="mis",
                                          name="fps")
                          for hp in range(HPC):
                              nc.tensor.matmul(
                                  pps2[:],
                                  lhsT=otts[hp][:, sj, :],
                                  rhs=wo_sb[:, hp, nt * QTW:(nt + 1) * QTW],
                                  start=(hp == 0), stop=(hp == HPC - 1))
                          if nt:
                              nc.scalar.copy(po[:, nt, :], pps2[:])
                          else:
                              nc.vector.tensor_copy(po[:, nt, :], pps2[:])
                      nc.sync.dma_start(
                          outp[s * P:(s + 1) * P, :], po[:])
    nc.compile()
    return nc


def _host_prep(inputs):
    q = np.asarray(inputs["query"], np.float32)
    k = np.asarray(inputs["key"], np.float32)
    v = np.asarray(inputs["value"], np.float32)
    mask = np.asarray(inputs["mask"])
    Wq = np.asarray(inputs["Wq"], np.float32)
    Wk = np.asarray(inputs["Wk"], np.float32)
    Wv = np.asarray(inputs["Wv"], np.float32)
    Wo = np.asarray(inputs["Wo"], np.float32)
    bq = np.asarray(inputs["bq"], np.float32)
    bk = np.asarray(inputs["bk"], np.float32)
    bv = np.asarray(inputs["bv"], np.float32)
    bo = np.asarray(inputs["bo"], np.float32)
    wgq = np.asarray(inputs["wgq"], np.float32)
    wgk = np.asarray(inputs["wgk"], np.float32)
    gb = float(np.asarray(inputs["gb"]))

    st, m01_b, midx = _prep_mask(mask)

    # x in [P, ND, S] batched-DMA layout
    xt_b = [[np.ascontiguousarray(
                x[b].T.reshape(ND, P, S).transpose(1, 0, 2)).astype(bf16)
             for b in range(B)] for x in (q, k, v)]

    def wslice(W, cols, scale=1.0):
        # W.T column slice [D, 128] -> [128, 8, 128] -> [128, 1024]
        wt = (W.T[:, cols:cols + CW] * scale).astype(bf16)
        return np.ascontiguousarray(
            wt.reshape(ND, P, CW).transpose(1, 0, 2).reshape(P, D))

    # q is pre-scaled by 0.5/sqrt(dk): scores arrive as s/2, and the
    # gate multiply computes (tanh(l/2)+1)*(s/2) = sigma(l)*s.
    qscale = 0.5 / np.sqrt(DK)
    ident = np.eye(P, dtype=bf16)

    meta = {
        "st": st, "midx": midx, "n_m01": m01_b[0].shape[1], "gb": gb,
        "use_bq": bool(np.any(bq)), "use_bk": bool(np.any(bk)),
        "use_bv": bool(np.any(bv)),
    }

    # gate weight row layouts: gq lands at rows h*DK of ug2 (compensated
    # for the q pre-scale); gkT lands at rows h*DK+1 of lk2.
    # ebias = -gb - ln2 for the u-exp; gb05 = gb/2 for the tanh bias
    ebias_h = np.full((P, 1), -gb - LN2, np.float32)
    gb05_h = np.full((P, 1), 0.5 * gb, np.float32)

    wgq_bd = np.zeros((P, DKP), np.float32)
    wgk_bd = np.zeros((P, HPC), np.float32)
    for h in range(HPC):
        wgq_bd[h * DK:(h + 1) * DK, h * DK] = wgq / qscale
        wgk_bd[h * DK:(h + 1) * DK, h] = wgk

    ngrp = NCORES // B          # head-groups per batch
    in_maps = []
    for c in range(NCORES):
        bc = c // ngrp          # batch of this core
        hg = c % ngrp           # head-group
        cols = [(hg * HPC + 0) * CW, (hg * HPC + 1) * CW]
        im = {
            "xqt": xt_b[0][bc], "xkt": xt_b[1][bc], "xvt": xt_b[2][bc],
            "wqt": np.ascontiguousarray(np.stack(
                [wslice(Wq, cl, qscale) for cl in cols]).transpose(1, 0, 2)),
            "wkt": np.ascontiguousarray(np.stack(
                [wslice(Wk, cl) for cl in cols]).transpose(1, 0, 2)),
            "wvt": np.ascontiguousarray(np.stack(
                [wslice(Wv, cl) for cl in cols]).transpose(1, 0, 2)),
            "wot": np.ascontiguousarray(np.stack(
                [Wo.T[cl:cl + CW, :].astype(bf16)
                 for cl in cols]).transpose(1, 0, 2)),
            "wgq": wgq_bd.astype(bf16), "wgk": wgk_bd.astype(bf16),
            "ebiasd": ebias_h, "gb05d": gb05_h,
            "identd": ident, "m01d": m01_b[bc],
            "onesd": np.ones((DK + 1, P), bf16),
            "bqt": np.ascontiguousarray(np.stack(
                [(bq[cl:cl + CW] * qscale).reshape(P, 1).astype(np.float32)
                 for cl in cols]).transpose(1, 0, 2)),
            "bkt": np.ascontiguousarray(np.stack(
                [bk[cl:cl + CW].reshape(P, 1).astype(np.float32)
                 for cl in cols]).transpose(1, 0, 2)),
            "bvt": np.ascontiguousarray(np.stack(
                [np.tile(bv[cl:cl + CW], (P, 1)).astype(np.float32)
                 for cl in cols]).transpose(1, 0, 2)),
        }
        in_maps.append(im)
    return meta, in_maps, bo


def kernel(**inputs):
    meta, in_maps, bo = _host_prep(inputs)

    key = (meta["st"].tobytes(), meta["gb"], meta["use_bq"],
           meta["use_bk"], meta["use_bv"], meta["n_m01"])
    if key not in _CACHE:
        _CACHE[key] = _build(meta)
    nc = _CACHE[key]

    from concourse.bass_utils import run_bass_kernel_spmd
    res = run_bass_kernel_spmd(
        nc, in_maps, core_ids=list(range(NCORES)),
        trace=bool(int(os.environ.get("KERNEL_TRACE", "0"))))
    out = np.zeros((B, S, D), np.float32)
    ngrp = NCORES // B
    for c, r in enumerate(res.results):
        out[c // ngrp] += r["outp"].astype(np.float32)
    out += bo
    if res.exec_time_ns is not None:
        print(f"HW exec time: {res.exec_time_ns} ns")
    return out


# revision 24
# speedup vs baseline: 1.0055x; 1.0055x over previous
"""Multi-head forgetting attention on 8 trn2 cores.

Sharding: 4 heads per core as 2 partition-slices ("hp") of 2 heads each
(head/tensor parallel). Each core receives the full (host-pre-transposed)
activations, its column slice of Wq/Wk/Wv, its row slice of Wo^T, and
produces a partial (S,D) bf16 output (both hp accumulated on-chip) which
the host sums in f32 (+ bo).

Gate structure: l = gq(q)+gk(k)+gb is materialised per 128x512 tile by a
2-contraction PE matmul from row-layout gq / gk vectors (bf16, PSUM), so
the ACT tanh pass needs no per-head bias and handles both heads per
instruction: sigma*s = (tanh(l/2)+1) * (s/2), with the 1/2 folded into
the host-side q scaling. For the aq=3 chunks the gate instead goes
through an exact reciprocal path (den = 0.5+0.5*u(q)v(k) on Pool from
exp vectors, reciprocal_approx_fast + multiply on DVE) which moves work
off the ACT engine. exp stays on ACT; the elementwise multiply reads
scores straight from PSUM (scores matmul emits bf16).

Other structure: flash-style streaming over k-tiles per 512-wide q chunk,
mixed 128x128 mask blocks multiplied by deduped 0/1 tiles (loaded once),
x tiles DMAed in 1MB batches, output projection accumulates both hp in
PSUM, one output DMA per 128-row slice.
"""

import os
import sys

sys.path.insert(0, "/opt/trn_rl_repo")

import numpy as np
import ml_dtypes

bf16 = ml_dtypes.bfloat16

B, S, D, H = 2, 2048, 1024, 16
DK = 64
NCORES = 8
HPC = 2          # head-pairs per core
CW = HPC * DK    # 128 per-slice head width
P = 128
QTW = 512        # q tile width (matmul free dim)
NQT = S // QTW   # 4
NKT = S // P     # 16 k tiles
NSL = S // P     # 16 q slices
ND = D // P      # 8 contraction tiles
DKP = DK + 2     # gq/gk row-layout tiles (2 rows per head-pair)
LN2 = 0.6931471805599453

# (aq, hp) chunks routed through the reciprocal gate path (off-ACT)
RECIP_PAIRS = {(3, 0), (3, 1)}

_CACHE = {}


def _prep_mask(mask):
    """Batch-union block table: 0 skip / 1 full / 2 mixed, plus per-batch
    0/1 tile contents for each union-mixed block (cores take their b's).
    Mixed tiles are deduped (consistently across batches) so each unique
    tile is loaded once."""
    m = np.asarray(mask).astype(bool)
    st = np.zeros((NKT, NSL), dtype=np.int8)
    uniq = {}
    m01 = [[] for _ in range(B)]
    midx = {}
    for i in range(NKT):
        for s in range(NSL):
            blks = [m[b, s * P:(s + 1) * P, i * P:(i + 1) * P]
                    for b in range(B)]
            alls = [blk.all() for blk in blks]
            anys = [blk.any() for blk in blks]
            if all(alls):
                st[i, s] = 1
            elif not any(anys):
                st[i, s] = 0
            else:
                st[i, s] = 2
                # used as matmul lhsT (applied transposed): keep blk
                # orientation so sc[k,q] += (blk[q,k]-1)*3e4
                tls = [np.ascontiguousarray(
                        (blks[b].astype(np.float32) - 1.0)
                        * 30000.0).astype(bf16) for b in range(B)]
                key = tuple(t.tobytes() for t in tls)
                if key not in uniq:
                    uniq[key] = len(m01[0])
                    for b in range(B):
                        m01[b].append(tls[b])
                midx[(i, s)] = uniq[key]
    if not m01[0]:
        for b in range(B):
            m01[b].append(np.zeros((P, P), dtype=bf16))
    # SBUF layout [P, n_uniq, P]
    return st, [np.stack(x, axis=1) for x in m01], midx


def _build(meta):
    """Build the (shared-across-cores) bass program."""
    import concourse.mybir as mybir
    import concourse.tile as tile
    from concourse import bacc

    st = meta["st"]
    midx = meta["midx"]
    n_m01 = meta["n_m01"]
    gb = meta["gb"]
    use_bq = meta["use_bq"]
    use_bk = meta["use_bk"]
    use_bv = meta["use_bv"]

    f32 = mybir.dt.float32
    b16 = mybir.dt.bfloat16
    Act = mybir.ActivationFunctionType
    Alu = mybir.AluOpType

    # block tables (batch-union)
    iv_qt = {qt: [i for i in range(NKT)
                  if any(st[i, 4 * qt + j] for j in range(4))]
             for qt in range(NQT)}
    valid_i = {s: [i for i in range(NKT) if st[i, s]]
               for s in range(NSL)}
    # attention chunk qt can only be emitted after projection chunk
    # mc[qt] (the latest chunk producing a k-tile it reads)
    mc = {qt: max((i // (QTW // P) for i in iv_qt[qt]), default=0)
          for qt in range(NQT)}

    nc = bacc.Bacc("TRN2", debug=False, enable_asserts=False,
                   num_devices=NCORES)

    xqt = nc.dram_tensor("xqt", (P, ND, S), b16, kind="ExternalInput")
    xkt = nc.dram_tensor("xkt", (P, ND, S), b16, kind="ExternalInput")
    xvt = nc.dram_tensor("xvt", (P, ND, S), b16, kind="ExternalInput")
    wqt = nc.dram_tensor("wqt", (P, HPC, D), b16, kind="ExternalInput")
    wkt = nc.dram_tensor("wkt", (P, HPC, D), b16, kind="ExternalInput")
    wvt = nc.dram_tensor("wvt", (P, HPC, D), b16, kind="ExternalInput")
    wot = nc.dram_tensor("wot", (P, HPC, D), b16, kind="ExternalInput")
    wgq = nc.dram_tensor("wgq", (P, DKP), b16, kind="ExternalInput")
    wgk = nc.dram_tensor("wgk", (P, HPC), b16, kind="ExternalInput")
    identd = nc.dram_tensor("identd", (P, P), b16, kind="ExternalInput")
    onesd = nc.dram_tensor("onesd", (DK + 1, P), b16, kind="ExternalInput")
    m01d = nc.dram_tensor("m01d", (P, n_m01, P), b16, kind="ExternalInput")
    ebiasd = nc.dram_tensor("ebiasd", (P, 1), f32, kind="ExternalInput")
    gb05d = nc.dram_tensor("gb05d", (P, 1), f32, kind="ExternalInput")
    bqt = nc.dram_tensor("bqt", (P, HPC, 1), f32, kind="ExternalInput")
    bkt = nc.dram_tensor("bkt", (P, HPC, 1), f32, kind="ExternalInput")
    bvt = nc.dram_tensor("bvt", (P, HPC, P), f32, kind="ExternalInput")
    outp = nc.dram_tensor("outp", (S, D), b16, kind="ExternalOutput")

    with tile.TileContext(nc) as tc:
        from contextlib import ExitStack
        with ExitStack() as ctx:
            cst = ctx.enter_context(tc.tile_pool(name="cst", bufs=1))
            per = ctx.enter_context(tc.tile_pool(name="per", bufs=1))
            strm = ctx.enter_context(tc.tile_pool(name="strm", bufs=1))
            work = ctx.enter_context(tc.tile_pool(name="work", bufs=2))
            prb = ctx.enter_context(tc.tile_pool(name="prb", bufs=1))
            mis = ctx.enter_context(
                tc.tile_pool(name="mis", bufs=2, space="PSUM"))
            scp = ctx.enter_context(
                tc.tile_pool(name="scp", bufs=5, space="PSUM"))
            att = ctx.enter_context(
                tc.tile_pool(name="att", bufs=1, space="PSUM"))

            # ---- constants ----
            wq_sb = cst.tile([P, HPC, D], b16, name="wq_sb")
            wk_sb = cst.tile([P, HPC, D], b16, name="wk_sb")
            wv_sb = cst.tile([P, HPC, D], b16, name="wv_sb")
            wo_sb = cst.tile([P, HPC, D], b16, name="wo_sb")
            wgq_sb = cst.tile([P, DKP], b16, name="wgq_sb")
            wgk_sb = cst.tile([P, HPC], b16, name="wgk_sb")
            id_sb = cst.tile([P, P], b16, name="id_sb")
            ones_sb = cst.tile([DK + 1, P], b16, name="ones_sb")
            m01_sb = cst.tile([P, n_m01, P], b16, name="m01_sb")
            ebias = cst.tile([P, 1], f32, name="ebias")
            gb05 = cst.tile([P, 1], f32, name="gb05")
            bq_sb = cst.tile([P, HPC, 1], f32, name="bq_sb")
            bk_sb = cst.tile([P, HPC, 1], f32, name="bk_sb")
            bv_sb = cst.tile([P, HPC, P], f32, name="bv_sb")
            nc.sync.dma_start(wq_sb[:], wqt[:, :, :])
            nc.sync.dma_start(wk_sb[:], wkt[:, :, :])
            nc.sync.dma_start(wv_sb[:], wvt[:, :, :])
            nc.sync.dma_start(wo_sb[:], wot[:, :, :])
            nc.sync.dma_start(wgq_sb[:], wgq[:, :])
            nc.sync.dma_start(wgk_sb[:], wgk[:, :])
            nc.sync.dma_start(id_sb[:], identd[:, :])
            nc.sync.dma_start(ones_sb[:], onesd[:, :])
            nc.scalar.dma_start(m01_sb[:], m01d[:, :, :])
            nc.sync.dma_start(ebias[:], ebiasd[:, :])
            nc.sync.dma_start(gb05[:], gb05d[:, :])
            if use_bq:
                nc.sync.dma_start(bq_sb[:], bqt[:, :, :])
            if use_bk:
                nc.sync.dma_start(bk_sb[:], bkt[:, :, :])
            if use_bv:
                nc.sync.dma_start(bv_sb[:], bvt[:, :, :])

            # per-hp persistent buffers
            qt_sb = [per.tile([P, S], b16, name=f"qt{hp}_sb", tag=f"qt{hp}")
                     for hp in range(HPC)]
            kt_sb = [per.tile([P, S], b16, name=f"kt{hp}_sb", tag=f"kt{hp}")
                     for hp in range(HPC)]
            v2_sb = [per.tile([P, NKT, HPC, DK + 1], b16,
                              name=f"v{hp}_sb", tag=f"v{hp}")
                     for hp in range(HPC)]
            ug2_sb = [per.tile([DKP, S], b16, name=f"ug{hp}_sb",
                               tag=f"ug{hp}") for hp in range(HPC)]
            gqb_sb = [per.tile([P, HPC, S], b16, name=f"gqb{hp}_sb",
                               tag=f"gqb{hp}") for hp in range(HPC)]
            ubc_sb = [per.tile([P, HPC, S], b16, name=f"ubc{hp}_sb",
                               tag=f"ubc{hp}") for hp in range(HPC)]
            gk05_sb = [per.tile([P, HPC, NKT], f32, name=f"gk05{hp}_sb",
                                tag=f"gk05{hp}") for hp in range(HPC)]
            vex_sb = [per.tile([P, HPC, NKT], f32, name=f"vex{hp}_sb",
                               tag=f"vex{hp}") for hp in range(HPC)]

            for hp in range(HPC):
                nc.vector.memset(v2_sb[hp][:, :, :, DK], 1.0)

            for qt in range(NQT):
                # ===== projection chunk qt (x tiles shared by both hp) ====
                q0 = qt * QTW
                # Q / K chunk qt: one batched x tile feeds both hp matmuls
                for (xsrc, wsb, osb, bias_sb, use_b, dmae, xtag) in (
                        (xqt, wq_sb, qt_sb, bq_sb, use_bq, nc.scalar, "xq"),
                        (xkt, wk_sb, kt_sb, bk_sb, use_bk, nc.sync, "xk")):
                    xt = strm.tile([P, ND, QTW], b16, tag=xtag, name="xt")
                    dmae.dma_start(xt[:], xsrc[:, :, q0:q0 + QTW])
                    pps = [scp.tile([P, QTW], f32, tag="sc",
                                    name=f"pps{hp}") for hp in range(HPC)]
                    for dt in range(ND):
                        for hp in range(HPC):
                            nc.tensor.matmul(
                                pps[hp][:],
                                lhsT=wsb[:, hp, dt * P:(dt + 1) * P],
                                rhs=xt[:, dt, :],
                                start=(dt == 0), stop=(dt == ND - 1))
                    for hp in range(HPC):
                        dst = osb[hp][:, q0:q0 + QTW]
                        if use_b:
                            nc.scalar.activation(
                                dst, pps[hp][:], Act.Identity,
                                bias=bias_sb[:, hp, :])
                        elif hp:
                            nc.scalar.copy(dst, pps[hp][:])
                        else:
                            nc.vector.tensor_copy(dst, pps[hp][:])

                # V slices 4qt..4qt+3
                xv = strm.tile([P, ND, QTW], b16, tag="xv", name="xv")
                nc.sync.dma_start(xv[:], xvt[:, :, q0:q0 + QTW])
                for hp in range(HPC):
                    for sj in range(QTW // P):
                        sl = qt * (QTW // P) + sj
                        vps = scp.tile([P, HPC, DK], f32, tag="sc",
                                       name="vps")
                        for dt in range(ND):
                            nc.tensor.matmul(
                                vps[:],
                                lhsT=xv[:, dt, sj * P:(sj + 1) * P],
                                rhs=wv_sb[:, hp, dt * P:(dt + 1) * P],
                                start=(dt == 0), stop=(dt == ND - 1))
                        if use_bv:
                            for h in range(HPC):
                                nc.vector.tensor_add(
                                    vps[:, h, :], vps[:, h, :],
                                    bv_sb[:, hp, h * DK:(h + 1) * DK])
                        # both heads in one strided copy
                        if sj % 2:
                            nc.scalar.copy(
                                v2_sb[hp][:, sl, :, 0:DK], vps[:])
                        else:
                            nc.vector.tensor_copy(
                                v2_sb[hp][:, sl, :, 0:DK], vps[:])

                for hp in range(HPC):
                    # gq row chunk, then its partition-broadcast tiles
                    gps = mis.tile([DKP, QTW], f32, tag="mis", name="gps")
                    nc.tensor.matmul(
                        gps[:], lhsT=wgq_sb[:],
                        rhs=qt_sb[hp][:, q0:q0 + QTW],
                        start=True, stop=True)
                    nc.scalar.copy(ug2_sb[hp][:, q0:q0 + QTW], gps[:])
                    for h in range(HPC):
                        gqp = mis.tile([P, QTW], f32, tag="mis",
                                       name="gqp")
                        nc.tensor.matmul(
                            gqp[:],
                            lhsT=ones_sb[h * DK:h * DK + 1, :],
                            rhs=ug2_sb[hp][h * DK:h * DK + 1,
                                           q0:q0 + QTW],
                            start=True, stop=True)
                        if h:
                            nc.scalar.copy(
                                gqb_sb[hp][:, h, q0:q0 + QTW], gqp[:])
                        else:
                            nc.vector.tensor_copy(
                                gqb_sb[hp][:, h, q0:q0 + QTW], gqp[:])
                        nc.scalar.activation(
                            ubc_sb[hp][:, h, q0:q0 + QTW], gqp[:],
                            Act.Exp, scale=-1.0, bias=ebias[:, :])

                    # per-partition gk: gk05 (tanh bias) + vex (recip)
                    for h in range(HPC):
                        hsl = slice(h * DK, (h + 1) * DK)
                        gkp = mis.tile([P, QTW // P], f32, tag="mis",
                                       name="gkp")
                        for j in range(QTW // P):
                            i = qt * (QTW // P) + j
                            nc.tensor.matmul(
                                gkp[:, j:j + 1],
                                lhsT=kt_sb[hp][hsl, i * P:(i + 1) * P],
                                rhs=wgk_sb[hsl, h:h + 1],
                                start=(j == 0), stop=(j == QTW // P - 1),
                                skip_group_check=True)
                        nc.scalar.activation(
                            gk05_sb[hp][:, h, qt * (QTW // P):
                                        (qt + 1) * (QTW // P)],
                            gkp[:], Act.Identity, scale=0.5,
                            bias=gb05[:, :])
                        nc.scalar.activation(
                            vex_sb[hp][:, h, qt * (QTW // P):
                                       (qt + 1) * (QTW // P)],
                            gkp[:], Act.Exp, scale=-1.0)

                # ===== attention chunks whose k-tiles are now ready ====
                ready = [aq for aq in range(NQT)
                         if mc[aq] == qt or (qt == NQT - 1 and mc[aq] > qt)]
                for aq in ready:
                  a0 = aq * QTW
                  otts = {}
                  for hp in range(HPC):
                      probs = {}
                      tiles = []
                      for n_i, i in enumerate(iv_qt[aq]):
                          recip = (n_i % 3 == 1)
                          # first non-skip 128-slice of this (i, aq)
                          sjlo = min(j for j in range(QTW // P)
                                     if st[i, aq * (QTW // P) + j])
                          tiles.append((i, recip, sjlo * P))
                      # pre-emit gate precursors (tanh on ACT, den on
                      # Pool): all inputs are chunk-level, so these run
                      # ahead of the score pipeline without stalls
                      pre = {}
                      for (i, recip, off) in tiles:
                          for h in range(HPC):
                              if recip:
                                  dn = work.tile([P, QTW], f32,
                                                 tag="den", name="den",
                                                 bufs=4)
                                  nc.gpsimd.tensor_scalar(
                                      dn[:, off:],
                                      ubc_sb[hp][:, h, a0 + off:a0 + QTW],
                                      vex_sb[hp][:, h, i:i + 1], 0.5,
                                      Alu.mult, Alu.add)
                                  pre[(i, h)] = dn
                              else:
                                  tnh = work.tile([P, QTW], b16,
                                                  tag="tnh", name="tnh",
                                                  bufs=16)
                                  nc.scalar.activation(
                                      tnh[:, off:],
                                      gqb_sb[hp][:, h, a0 + off:a0 + QTW],
                                      Act.Tanh, scale=0.5,
                                      bias=gk05_sb[hp][:, h, i:i + 1])
                                  pre[(i, h)] = tnh
                      for (i, recip, off) in tiles:
                          p3 = prb.tile([P, HPC, QTW], b16, tag=f"pr{i}",
                                        name=f"pr{i}",
                                        bufs=2 if i < 8 else 1)
                          g3 = work.tile([P, HPC, QTW], b16, tag="gat",
                                         name="gat")
                          sjlo = off // P
                          mixed = [sj for sj in range(sjlo, QTW // P)
                                   if st[i, aq * (QTW // P) + sj] == 2]
                          for h in range(HPC):
                              hsl = slice(h * DK, (h + 1) * DK)
                              sch = scp.tile([P, QTW], f32, tag="sc",
                                             name="sch")
                              nc.tensor.matmul(
                                  sch[:, off:],
                                  lhsT=kt_sb[hp][hsl, i * P:(i + 1) * P],
                                  rhs=qt_sb[hp][hsl, a0 + off:a0 + QTW],
                                  start=True, stop=not mixed)
                              # additive mask bias: sc += m01^T . I
                              for n, sj in enumerate(mixed):
                                  s = aq * (QTW // P) + sj
                                  nc.tensor.matmul(
                                      sch[:, sj * P:(sj + 1) * P],
                                      lhsT=m01_sb[:, midx[(i, s)], :],
                                      rhs=id_sb[:],
                                      start=False,
                                      stop=(n == len(mixed) - 1),
                                      skip_group_check=True)
                              if recip:
                                  rc = work.tile([P, QTW], f32, tag="rec",
                                                 name="rec", bufs=3)
                                  nc.vector.reciprocal_approx_fast(
                                      rc[:, off:], pre[(i, h)][:, off:])
                                  nc.vector.tensor_tensor(
                                      g3[:, h, off:], sch[:, off:],
                                      rc[:, off:], Alu.mult)
                              else:
                                  nc.vector.scalar_tensor_tensor(
                                      g3[:, h, off:], pre[(i, h)][:, off:],
                                      1.0, sch[:, off:],
                                      Alu.add, Alu.mult)
                          nc.scalar.activation(
                              p3[:, :, off:], g3[:, :, off:], Act.Exp)
                          probs[i] = p3

                      # attn @ V, normalize, transpose; out-proj deferred
                      ott = work.tile([P, QTW // P, P], b16,
                                      tag=f"ott{hp}", name=f"ott{hp}")
                      otts[hp] = ott
                      for sj in range(QTW // P):
                          s = aq * (QTW // P) + sj
                          onat = work.tile([P, P], b16, tag="onat",
                                           name="onat")
                          ops = att.tile([P, HPC * (DK + 1)], f32, tag="o",
                                         name="ops")
                          vi = valid_i[s]
                          for h in range(HPC):
                              ob = h * (DK + 1)
                              if not vi:
                                  nc.vector.memset(
                                      ops[:, ob:ob + DK + 1], 0.0)
                              for n, i in enumerate(vi):
                                  nc.tensor.matmul(
                                      ops[:, ob:ob + DK + 1],
                                      lhsT=probs[i][:, h,
                                                    sj * P:(sj + 1) * P],
                                      rhs=v2_sb[hp][:, i, h, :],
                                      start=(n == 0),
                                      stop=(n == len(vi) - 1),
                                      skip_group_check=True)
                              recv = work.tile([P, 1], f32, tag="recip",
                                               name="recip", bufs=4)
                              nc.vector.reciprocal_approx_fast(
                                  recv[:], ops[:, ob + DK:ob + DK + 1])
                              nc.vector.tensor_scalar_mul(
                                  onat[:, h * DK:(h + 1) * DK],
                                  ops[:, ob:ob + DK], recv[:])
                          trp = mis.tile([P, P], b16, tag="mis", name="trp")
                          nc.tensor.transpose(trp[:], onat[:], id_sb[:])
                          nc.vector.tensor_copy(ott[:, sj, :], trp[:])
                  # output projection: accumulate both hp per s-slice
                  for sj in range(QTW // P):
                      s = aq * (QTW // P) + sj
                      po = work.tile([P, 2, QTW], b16, tag="po",
                                     name="po", bufs=3)
                      for nt in range(2):
                          pps2 = mis.tile([P, QTW], f32, tag="mis",
                                          name="fps")
                          for hp in range(HPC):
                              nc.tensor.matmul(
                                  pps2[:],
                                  lhsT=otts[hp][:, sj, :],
                                  rhs=wo_sb[:, hp, nt * QTW:(nt + 1) * QTW],
                                  start=(hp == 0), stop=(hp == HPC - 1))
                          if nt:
                              nc.scalar.copy(po[:, nt, :], pps2[:])
                          else:
                              nc.vector.tensor_copy(po[:, nt, :], pps2[:])
                      nc.sync.dma_start(
                          outp[s * P:(s + 1) * P, :], po[:])
    nc.compile()
    return nc


def _host_prep(inputs):
    q = np.asarray(inputs["query"], np.float32)
    k = np.asarray(inputs["key"], np.float32)
    v = np.asarray(inputs["value"], np.float32)
    mask = np.asarray(inputs["mask"])
    Wq = np.asarray(inputs["Wq"], np.float32)
    Wk = np.asarray(inputs["Wk"], np.float32)
    Wv = np.asarray(inputs["Wv"], np.float32)
    Wo = np.asarray(inputs["Wo"], np.float32)
    bq = np.asarray(inputs["bq"], np.float32)
    bk = np.asarray(inputs["bk"], np.float32)
    bv = np.asarray(inputs["bv"], np.float32)
    bo = np.asarray(inputs["bo"], np.float32)
    wgq = np.asarray(inputs["wgq"], np.float32)
    wgk = np.asarray(inputs["wgk"], np.float32)
    gb = float(np.asarray(inputs["gb"]))

    st, m01_b, midx = _prep_mask(mask)

    # x in [P, ND, S] batched-DMA layout
    xt_b = [[np.ascontiguousarray(
                x[b].T.reshape(ND, P, S).transpose(1, 0, 2)).astype(bf16)
             for b in range(B)] for x in (q, k, v)]

    def wslice(W, cols, scale=1.0):
        # W.T column slice [D, 128] -> [128, 8, 128] -> [128, 1024]
        wt = (W.T[:, cols:cols + CW] * scale).astype(bf16)
        return np.ascontiguousarray(
            wt.reshape(ND, P, CW).transpose(1, 0, 2).reshape(P, D))

    # q is pre-scaled by 0.5/sqrt(dk): scores arrive as s/2, and the
    # gate multiply computes (tanh(l/2)+1)*(s/2) = sigma(l)*s.
    qscale = 0.5 / np.sqrt(DK)
    ident = np.eye(P, dtype=bf16)

    meta = {
        "st": st, "midx": midx, "n_m01": m01_b[0].shape[1], "gb": gb,
        "use_bq": bool(np.any(bq)), "use_bk": bool(np.any(bk)),
        "use_bv": bool(np.any(bv)),
    }

    # gate weight row layouts: gq lands at rows h*DK of ug2 (compensated
    # for the q pre-scale); gkT lands at rows h*DK+1 of lk2.
    # ebias = -gb - ln2 for the u-exp; gb05 = gb/2 for the tanh bias
    ebias_h = np.full((P, 1), -gb - LN2, np.float32)
    gb05_h = np.full((P, 1), 0.5 * gb, np.float32)

    wgq_bd = np.zeros((P, DKP), np.float32)
    wgk_bd = np.zeros((P, HPC), np.float32)
    for h in range(HPC):
        wgq_bd[h * DK:(h + 1) * DK, h * DK] = wgq / qscale
        wgk_bd[h * DK:(h + 1) * DK, h] = wgk

    ngrp = NCORES // B          # head-groups per batch
    in_maps = []
    for c in range(NCORES):
        bc = c // ngrp          # batch of this core
        hg = c % ngrp           # head-group
        cols = [(hg * HPC + 0) * CW, (hg * HPC + 1) * CW]
        im = {
            "xqt": xt_b[0][bc], "xkt": xt_b[1][bc], "xvt": xt_b[2][bc],
            "wqt": np.ascontiguousarray(np.stack(
                [wslice(Wq, cl, qscale) for cl in cols]).transpose(1, 0, 2)),
            "wkt": np.ascontiguousarray(np.stack(
                [wslice(Wk, cl) for cl in cols]).transpose(1, 0, 2)),
            "wvt": np.ascontiguousarray(np.stack(
                [wslice(Wv, cl) for cl in cols]).transpose(1, 0, 2)),
            "wot": np.ascontiguousarray(np.stack(
                [Wo.T[cl:cl + CW, :].astype(bf16)
                 for cl in cols]).transpose(1, 0, 2)),
            "wgq": wgq_bd.astype(bf16), "wgk": wgk_bd.astype(bf16),
            "ebiasd": ebias_h, "gb05d": gb05_h,
            "identd": ident, "m01d": m01_b[bc],
            "onesd": np.ones((DK + 1, P), bf16),
            "bqt": np.ascontiguousarray(np.stack(
                [(bq[cl:cl + CW] * qscale).reshape(P, 1).astype(np.float32)
                 for cl in cols]).transpose(1, 0, 2)),
            "bkt": np.ascontiguousarray(np.stack(
                [bk[cl:cl + CW].reshape(P, 1).astype(np.float32)
                 for cl in cols]).transpose(1, 0, 2)),
            "bvt": np.ascontiguousarray(np.stack(
                [np.tile(bv[cl:cl + CW], (P, 1)).astype(np.float32)
                 for cl in cols]).transpose(1, 0, 2)),
        }
        in_maps.append(im)
    return meta, in_maps, bo


def kernel(**inputs):
    meta, in_maps, bo = _host_prep(inputs)

    key = (meta["st"].tobytes(), meta["gb"], meta["use_bq"],
           meta["use_bk"], meta["use_bv"], meta["n_m01"])
    if key not in _CACHE:
        _CACHE[key] = _build(meta)
    nc = _CACHE[key]

    from concourse.bass_utils import run_bass_kernel_spmd
    res = run_bass_kernel_spmd(
        nc, in_maps, core_ids=list(range(NCORES)),
        trace=bool(int(os.environ.get("KERNEL_TRACE", "0"))))
    out = np.zeros((B, S, D), np.float32)
    ngrp = NCORES // B
    for c, r in enumerate(res.results):
        out[c // ngrp] += r["outp"].astype(np.float32)
    out += bo
    if res.exec_time_ns is not None:
        print(f"HW exec time: {res.exec_time_ns} ns")
    return out


# revision 25
# speedup vs baseline: 1.0642x; 1.0583x over previous
"""Multi-head forgetting attention on 8 trn2 cores.

Sharding: 4 heads per core as 2 partition-slices ("hp") of 2 heads each
(head/tensor parallel). Each core receives the full (host-pre-transposed)
activations, its column slice of Wq/Wk/Wv, its row slice of Wo^T, and
produces a partial (S,D) bf16 output (both hp accumulated on-chip) which
the host sums in f32 (+ bo).

Gate structure: l = gq(q)+gk(k)+gb is materialised per 128x512 tile by a
2-contraction PE matmul from row-layout gq / gk vectors (bf16, PSUM), so
the ACT tanh pass needs no per-head bias and handles both heads per
instruction: sigma*s = (tanh(l/2)+1) * (s/2), with the 1/2 folded into
the host-side q scaling. For the aq=3 chunks the gate instead goes
through an exact reciprocal path (den = 0.5+0.5*u(q)v(k) on Pool from
exp vectors, reciprocal_approx_fast + multiply on DVE) which moves work
off the ACT engine. exp stays on ACT; the elementwise multiply reads
scores straight from PSUM (scores matmul emits bf16).

Other structure: flash-style streaming over k-tiles per 512-wide q chunk,
mixed 128x128 mask blocks multiplied by deduped 0/1 tiles (loaded once),
x tiles DMAed in 1MB batches, output projection accumulates both hp in
PSUM, one output DMA per 128-row slice.
"""

import os
import sys

sys.path.insert(0, "/opt/trn_rl_repo")

import numpy as np
import ml_dtypes

bf16 = ml_dtypes.bfloat16

B, S, D, H = 2, 2048, 1024, 16
DK = 64
NCORES = 8
HPC = 2          # head-pairs per core
CW = HPC * DK    # 128 per-slice head width
P = 128
QTW = 512        # q tile width (matmul free dim)
NQT = S // QTW   # 4
NKT = S // P     # 16 k tiles
NSL = S // P     # 16 q slices
ND = D // P      # 8 contraction tiles
DKP = DK + 2     # gq/gk row-layout tiles (2 rows per head-pair)
LN2 = 0.6931471805599453

# (aq, hp) chunks routed through the reciprocal gate path (off-ACT)
RECIP_PAIRS = {(3, 0), (3, 1)}

_CACHE = {}


def _prep_mask(mask):
    """Batch-union block table: 0 skip / 1 full / 2 mixed, plus per-batch
    0/1 tile contents for each union-mixed block (cores take their b's).
    Mixed tiles are deduped (consistently across batches) so each unique
    tile is loaded once."""
    m = np.asarray(mask).astype(bool)
    st = np.zeros((NKT, NSL), dtype=np.int8)
    uniq = {}
    m01 = [[] for _ in range(B)]
    midx = {}
    for i in range(NKT):
        for s in range(NSL):
            blks = [m[b, s * P:(s + 1) * P, i * P:(i + 1) * P]
                    for b in range(B)]
            alls = [blk.all() for blk in blks]
            anys = [blk.any() for blk in blks]
            if all(alls):
                st[i, s] = 1
            elif not any(anys):
                st[i, s] = 0
            else:
                st[i, s] = 2
                # used as matmul lhsT (applied transposed): keep blk
                # orientation so sc[k,q] += (blk[q,k]-1)*3e4
                tls = [np.ascontiguousarray(
                        (blks[b].astype(np.float32) - 1.0)
                        * 30000.0).astype(bf16) for b in range(B)]
                key = tuple(t.tobytes() for t in tls)
                if key not in uniq:
                    uniq[key] = len(m01[0])
                    for b in range(B):
                        m01[b].append(tls[b])
                midx[(i, s)] = uniq[key]
    if not m01[0]:
        for b in range(B):
            m01[b].append(np.zeros((P, P), dtype=bf16))
    # SBUF layout [P, n_uniq, P]
    return st, [np.stack(x, axis=1) for x in m01], midx


def _build(meta):
    """Build the (shared-across-cores) bass program."""
    import concourse.mybir as mybir
    import concourse.tile as tile
    from concourse import bacc

    st = meta["st"]
    midx = meta["midx"]
    n_m01 = meta["n_m01"]
    gb = meta["gb"]
    use_bq = meta["use_bq"]
    use_bk = meta["use_bk"]
    use_bv = meta["use_bv"]

    f32 = mybir.dt.float32
    b16 = mybir.dt.bfloat16
    Act = mybir.ActivationFunctionType
    Alu = mybir.AluOpType

    # block tables (batch-union)
    iv_qt = {qt: [i for i in range(NKT)
                  if any(st[i, 4 * qt + j] for j in range(4))]
             for qt in range(NQT)}
    valid_i = {s: [i for i in range(NKT) if st[i, s]]
               for s in range(NSL)}
    # attention chunk qt can only be emitted after projection chunk
    # mc[qt] (the latest chunk producing a k-tile it reads)
    mc = {qt: max((i // (QTW // P) for i in iv_qt[qt]), default=0)
          for qt in range(NQT)}

    nc = bacc.Bacc("TRN2", debug=False, enable_asserts=False,
                   num_devices=NCORES)

    xqt = nc.dram_tensor("xqt", (P, ND, S), b16, kind="ExternalInput")
    xkt = nc.dram_tensor("xkt", (P, ND, S), b16, kind="ExternalInput")
    xvt = nc.dram_tensor("xvt", (P, ND, S), b16, kind="ExternalInput")
    wqt = nc.dram_tensor("wqt", (P, HPC, D), b16, kind="ExternalInput")
    wkt = nc.dram_tensor("wkt", (P, HPC, D), b16, kind="ExternalInput")
    wvt = nc.dram_tensor("wvt", (P, HPC, D), b16, kind="ExternalInput")
    wot = nc.dram_tensor("wot", (P, HPC, D), b16, kind="ExternalInput")
    wgq = nc.dram_tensor("wgq", (P, DKP), b16, kind="ExternalInput")
    wgk = nc.dram_tensor("wgk", (P, HPC), b16, kind="ExternalInput")
    identd = nc.dram_tensor("identd", (P, P), b16, kind="ExternalInput")
    onesd = nc.dram_tensor("onesd", (DK + 1, P), b16, kind="ExternalInput")
    m01d = nc.dram_tensor("m01d", (P, n_m01, P), b16, kind="ExternalInput")
    ebiasd = nc.dram_tensor("ebiasd", (P, 1), f32, kind="ExternalInput")
    gb05d = nc.dram_tensor("gb05d", (P, 1), f32, kind="ExternalInput")
    bqt = nc.dram_tensor("bqt", (P, HPC, 1), f32, kind="ExternalInput")
    bkt = nc.dram_tensor("bkt", (P, HPC, 1), f32, kind="ExternalInput")
    bvt = nc.dram_tensor("bvt", (P, HPC, P), f32, kind="ExternalInput")
    outp = nc.dram_tensor("outp", (S, D), b16, kind="ExternalOutput")

    with tile.TileContext(nc) as tc:
        from contextlib import ExitStack
        with ExitStack() as ctx:
            cst = ctx.enter_context(tc.tile_pool(name="cst", bufs=1))
            per = ctx.enter_context(tc.tile_pool(name="per", bufs=1))
            strm = ctx.enter_context(tc.tile_pool(name="strm", bufs=1))
            work = ctx.enter_context(tc.tile_pool(name="work", bufs=2))
            prb = ctx.enter_context(tc.tile_pool(name="prb", bufs=1))
            mis = ctx.enter_context(
                tc.tile_pool(name="mis", bufs=2, space="PSUM"))
            scp = ctx.enter_context(
                tc.tile_pool(name="scp", bufs=5, space="PSUM"))
            att = ctx.enter_context(
                tc.tile_pool(name="att", bufs=1, space="PSUM"))

            # ---- constants ----
            wq_sb = cst.tile([P, HPC, D], b16, name="wq_sb")
            wk_sb = cst.tile([P, HPC, D], b16, name="wk_sb")
            wv_sb = cst.tile([P, HPC, D], b16, name="wv_sb")
            wo_sb = cst.tile([P, HPC, D], b16, name="wo_sb")
            wgq_sb = cst.tile([P, DKP], b16, name="wgq_sb")
            wgk_sb = cst.tile([P, HPC], b16, name="wgk_sb")
            id_sb = cst.tile([P, P], b16, name="id_sb")
            ones_sb = cst.tile([DK + 1, P], b16, name="ones_sb")
            m01_sb = cst.tile([P, n_m01, P], b16, name="m01_sb")
            ebias = cst.tile([P, 1], f32, name="ebias")
            gb05 = cst.tile([P, 1], f32, name="gb05")
            bq_sb = cst.tile([P, HPC, 1], f32, name="bq_sb")
            bk_sb = cst.tile([P, HPC, 1], f32, name="bk_sb")
            bv_sb = cst.tile([P, HPC, P], f32, name="bv_sb")
            nc.sync.dma_start(wq_sb[:], wqt[:, :, :])
            nc.sync.dma_start(wk_sb[:], wkt[:, :, :])
            nc.sync.dma_start(wv_sb[:], wvt[:, :, :])
            nc.sync.dma_start(wo_sb[:], wot[:, :, :])
            nc.sync.dma_start(wgq_sb[:], wgq[:, :])
            nc.sync.dma_start(wgk_sb[:], wgk[:, :])
            nc.sync.dma_start(id_sb[:], identd[:, :])
            nc.sync.dma_start(ones_sb[:], onesd[:, :])
            nc.scalar.dma_start(m01_sb[:], m01d[:, :, :])
            nc.sync.dma_start(ebias[:], ebiasd[:, :])
            nc.sync.dma_start(gb05[:], gb05d[:, :])
            if use_bq:
                nc.sync.dma_start(bq_sb[:], bqt[:, :, :])
            if use_bk:
                nc.sync.dma_start(bk_sb[:], bkt[:, :, :])
            if use_bv:
                nc.sync.dma_start(bv_sb[:], bvt[:, :, :])

            # per-hp persistent buffers
            qt_sb = [per.tile([P, S], b16, name=f"qt{hp}_sb", tag=f"qt{hp}")
                     for hp in range(HPC)]
            kt_sb = [per.tile([P, S], b16, name=f"kt{hp}_sb", tag=f"kt{hp}")
                     for hp in range(HPC)]
            v2_sb = [per.tile([P, NKT, HPC, DK + 1], b16,
                              name=f"v{hp}_sb", tag=f"v{hp}")
                     for hp in range(HPC)]
            ug2_sb = [per.tile([DKP, S], b16, name=f"ug{hp}_sb",
                               tag=f"ug{hp}") for hp in range(HPC)]
            gqb_sb = [per.tile([P, HPC, S], b16, name=f"gqb{hp}_sb",
                               tag=f"gqb{hp}") for hp in range(HPC)]
            ubc_sb = [per.tile([P, HPC, S], b16, name=f"ubc{hp}_sb",
                               tag=f"ubc{hp}") for hp in range(HPC)]
            gk05_sb = [per.tile([P, HPC, NKT], f32, name=f"gk05{hp}_sb",
                                tag=f"gk05{hp}") for hp in range(HPC)]
            vex_sb = [per.tile([P, HPC, NKT], f32, name=f"vex{hp}_sb",
                               tag=f"vex{hp}") for hp in range(HPC)]

            for hp in range(HPC):
                nc.vector.memset(v2_sb[hp][:, :, :, DK], 1.0)

            for qt in range(NQT):
                # ===== projection chunk qt (x tiles shared by both hp) ====
                q0 = qt * QTW
                # Q / K chunk qt: one batched x tile feeds both hp matmuls
                for (xsrc, wsb, osb, bias_sb, use_b, dmae, xtag) in (
                        (xqt, wq_sb, qt_sb, bq_sb, use_bq, nc.scalar, "xq"),
                        (xkt, wk_sb, kt_sb, bk_sb, use_bk, nc.sync, "xk")):
                    xt = strm.tile([P, ND, QTW], b16, tag=xtag, name="xt")
                    dmae.dma_start(xt[:], xsrc[:, :, q0:q0 + QTW])
                    pps = [scp.tile([P, QTW], f32, tag="sc",
                                    name=f"pps{hp}") for hp in range(HPC)]
                    for dt in range(ND):
                        for hp in range(HPC):
                            nc.tensor.matmul(
                                pps[hp][:],
                                lhsT=wsb[:, hp, dt * P:(dt + 1) * P],
                                rhs=xt[:, dt, :],
                                start=(dt == 0), stop=(dt == ND - 1))
                    for hp in range(HPC):
                        dst = osb[hp][:, q0:q0 + QTW]
                        if use_b:
                            nc.scalar.activation(
                                dst, pps[hp][:], Act.Identity,
                                bias=bias_sb[:, hp, :])
                        elif hp:
                            nc.scalar.copy(dst, pps[hp][:])
                        else:
                            nc.vector.tensor_copy(dst, pps[hp][:])

                # V slices 4qt..4qt+3
                xv = strm.tile([P, ND, QTW], b16, tag="xv", name="xv")
                nc.sync.dma_start(xv[:], xvt[:, :, q0:q0 + QTW])
                for hp in range(HPC):
                    for sj in range(QTW // P):
                        sl = qt * (QTW // P) + sj
                        vps = scp.tile([P, HPC, DK], f32, tag="sc",
                                       name="vps")
                        for dt in range(ND):
                            nc.tensor.matmul(
                                vps[:],
                                lhsT=xv[:, dt, sj * P:(sj + 1) * P],
                                rhs=wv_sb[:, hp, dt * P:(dt + 1) * P],
                                start=(dt == 0), stop=(dt == ND - 1))
                        if use_bv:
                            for h in range(HPC):
                                nc.vector.tensor_add(
                                    vps[:, h, :], vps[:, h, :],
                                    bv_sb[:, hp, h * DK:(h + 1) * DK])
                        # both heads in one strided copy
                        if sj % 2:
                            nc.scalar.copy(
                                v2_sb[hp][:, sl, :, 0:DK], vps[:])
                        else:
                            nc.vector.tensor_copy(
                                v2_sb[hp][:, sl, :, 0:DK], vps[:])

                for hp in range(HPC):
                    # gq row chunk, then its partition-broadcast tiles
                    gps = mis.tile([DKP, QTW], f32, tag="mis", name="gps")
                    nc.tensor.matmul(
                        gps[:], lhsT=wgq_sb[:],
                        rhs=qt_sb[hp][:, q0:q0 + QTW],
                        start=True, stop=True)
                    nc.scalar.copy(ug2_sb[hp][:, q0:q0 + QTW], gps[:])
                    for h in range(HPC):
                        gqp = mis.tile([P, QTW], f32, tag="mis",
                                       name="gqp")
                        nc.tensor.matmul(
                            gqp[:],
                            lhsT=ones_sb[h * DK:h * DK + 1, :],
                            rhs=ug2_sb[hp][h * DK:h * DK + 1,
                                           q0:q0 + QTW],
                            start=True, stop=True)
                        if h:
                            nc.scalar.copy(
                                gqb_sb[hp][:, h, q0:q0 + QTW], gqp[:])
                        else:
                            nc.vector.tensor_copy(
                                gqb_sb[hp][:, h, q0:q0 + QTW], gqp[:])
                        nc.scalar.activation(
                            ubc_sb[hp][:, h, q0:q0 + QTW], gqp[:],
                            Act.Exp, scale=-1.0, bias=ebias[:, :])

                    # per-partition gk: gk05 (tanh bias) + vex (recip)
                    for h in range(HPC):
                        hsl = slice(h * DK, (h + 1) * DK)
                        gkp = mis.tile([P, QTW // P], f32, tag="mis",
                                       name="gkp")
                        for j in range(QTW // P):
                            i = qt * (QTW // P) + j
                            nc.tensor.matmul(
                                gkp[:, j:j + 1],
                                lhsT=kt_sb[hp][hsl, i * P:(i + 1) * P],
                                rhs=wgk_sb[hsl, h:h + 1],
                                start=(j == 0), stop=(j == QTW // P - 1),
                                skip_group_check=True)
                        nc.scalar.activation(
                            gk05_sb[hp][:, h, qt * (QTW // P):
                                        (qt + 1) * (QTW // P)],
                            gkp[:], Act.Identity, scale=0.5,
                            bias=gb05[:, :])
                        nc.scalar.activation(
                            vex_sb[hp][:, h, qt * (QTW // P):
                                       (qt + 1) * (QTW // P)],
                            gkp[:], Act.Exp, scale=-1.0)

                # ===== attention chunks whose k-tiles are now ready ====
                ready = [aq for aq in range(NQT)
                         if mc[aq] == qt or (qt == NQT - 1 and mc[aq] > qt)]
                for aq in ready:
                  a0 = aq * QTW
                  otts = {}
                  for hp in range(HPC):
                      probs = {}
                      tiles = []
                      for n_i, i in enumerate(iv_qt[aq]):
                          recip = (n_i % 3 == 1)
                          # first non-skip 128-slice of this (i, aq)
                          sjlo = min(j for j in range(QTW // P)
                                     if st[i, aq * (QTW // P) + j])
                          tiles.append((i, recip, sjlo * P))
                      for (i, recip, off) in tiles:
                          pre = {}
                          for h in range(HPC):
                              if recip:
                                  dn = work.tile([P, QTW], f32,
                                                 tag="den", name="den",
                                                 bufs=4)
                                  nc.gpsimd.tensor_scalar(
                                      dn[:, off:],
                                      ubc_sb[hp][:, h, a0 + off:a0 + QTW],
                                      vex_sb[hp][:, h, i:i + 1], 0.5,
                                      Alu.mult, Alu.add)
                                  pre[(i, h)] = dn
                              else:
                                  tnh = work.tile([P, QTW], b16,
                                                  tag="tnh", name="tnh",
                                                  bufs=6)
                                  nc.scalar.activation(
                                      tnh[:, off:],
                                      gqb_sb[hp][:, h, a0 + off:a0 + QTW],
                                      Act.Tanh, scale=0.5,
                                      bias=gk05_sb[hp][:, h, i:i + 1])
                                  pre[(i, h)] = tnh
                          p3 = prb.tile([P, HPC, QTW], b16, tag=f"pr{i}",
                                        name=f"pr{i}",
                                        bufs=2 if i < 8 else 1)
                          g3 = work.tile([P, HPC, QTW], b16, tag="gat",
                                         name="gat")
                          sjlo = off // P
                          mixed = [sj for sj in range(sjlo, QTW // P)
                                   if st[i, aq * (QTW // P) + sj] == 2]
                          for h in range(HPC):
                              hsl = slice(h * DK, (h + 1) * DK)
                              sch = scp.tile([P, QTW], f32, tag="sc",
                                             name="sch")
                              nc.tensor.matmul(
                                  sch[:, off:],
                                  lhsT=kt_sb[hp][hsl, i * P:(i + 1) * P],
                                  rhs=qt_sb[hp][hsl, a0 + off:a0 + QTW],
                                  start=True, stop=not mixed)
                              # additive mask bias: sc += m01^T . I
                              for n, sj in enumerate(mixed):
                                  s = aq * (QTW // P) + sj
                                  nc.tensor.matmul(
                                      sch[:, sj * P:(sj + 1) * P],
                                      lhsT=m01_sb[:, midx[(i, s)], :],
                                      rhs=id_sb[:],
                                      start=False,
                                      stop=(n == len(mixed) - 1),
                                      skip_group_check=True)
                              if recip:
                                  rc = work.tile([P, QTW], f32, tag="rec",
                                                 name="rec", bufs=3)
                                  nc.vector.reciprocal_approx_fast(
                                      rc[:, off:], pre[(i, h)][:, off:])
                                  nc.vector.tensor_tensor(
                                      g3[:, h, off:], sch[:, off:],
                                      rc[:, off:], Alu.mult)
                              else:
                                  nc.vector.scalar_tensor_tensor(
                                      g3[:, h, off:], pre[(i, h)][:, off:],
                                      1.0, sch[:, off:],
                                      Alu.add, Alu.mult)
                          nc.scalar.activation(
                              p3[:, :, off:], g3[:, :, off:], Act.Exp)
                          probs[i] = p3

                      # attn @ V, normalize, transpose; out-proj deferred
                      ott = work.tile([P, QTW // P, P], b16,
                                      tag=f"ott{hp}", name=f"ott{hp}")
                      otts[hp] = ott
                      for sj in range(QTW // P):
                          s = aq * (QTW // P) + sj
                          onat = work.tile([P, P], b16, tag="onat",
                                           name="onat")
                          ops = att.tile([P, HPC * (DK + 1)], f32, tag="o",
                                         name="ops")
                          vi = valid_i[s]
                          for h in range(HPC):
                              ob = h * (DK + 1)
                              if not vi:
                                  nc.vector.memset(
                                      ops[:, ob:ob + DK + 1], 0.0)
                              for n, i in enumerate(vi):
                                  nc.tensor.matmul(
                                      ops[:, ob:ob + DK + 1],
                                      lhsT=probs[i][:, h,
                                                    sj * P:(sj + 1) * P],
                                      rhs=v2_sb[hp][:, i, h, :],
                                      start=(n == 0),
                                      stop=(n == len(vi) - 1),
                                      skip_group_check=True)
                              recv = work.tile([P, 1], f32, tag="recip",
                                               name="recip", bufs=4)
                              nc.vector.reciprocal_approx_fast(
                                  recv[:], ops[:, ob + DK:ob + DK + 1])
                              nc.vector.tensor_scalar_mul(
                                  onat[:, h * DK:(h + 1) * DK],
                                  ops[:, ob:ob + DK], recv[:])
                          trp = mis.tile([P, P], b16, tag="mis", name="trp")
                          nc.tensor.transpose(trp[:], onat[:], id_sb[:])
                          nc.vector.tensor_copy(ott[:, sj, :], trp[:])
                  # output projection: accumulate both hp per s-slice
                  for sj in range(QTW // P):
                      s = aq * (QTW // P) + sj
                      po = work.tile([P, 2, QTW], b16, tag="po",
                                     name="po", bufs=3)
                      for nt in range(2):
                          pps2 = mis.tile([P, QTW], f32, tag="mis",
                                          name="fps")
                          for hp in range(HPC):
                              nc.tensor.matmul(
                                  pps2[:],
                                  lhsT=otts[hp][:, sj, :],
                                  rhs=wo_sb[:, hp, nt * QTW:(nt + 1) * QTW],
                                  start=(hp == 0), stop=(hp == HPC - 1))
                          if nt:
                              nc.scalar.copy(po[:, nt, :], pps2[:])
                          else:
                              nc.vector.tensor_copy(po[:, nt, :], pps2[:])
                      nc.sync.dma_start(
                          outp[s * P:(s + 1) * P, :], po[:])
    nc.compile()
    return nc


def _host_prep(inputs):
    q = np.asarray(inputs["query"], np.float32)
    k = np.asarray(inputs["key"], np.float32)
    v = np.asarray(inputs["value"], np.float32)
    mask = np.asarray(inputs["mask"])
    Wq = np.asarray(inputs["Wq"], np.float32)
    Wk = np.asarray(inputs["Wk"], np.float32)
    Wv = np.asarray(inputs["Wv"], np.float32)
    Wo = np.asarray(inputs["Wo"], np.float32)
    bq = np.asarray(inputs["bq"], np.float32)
    bk = np.asarray(inputs["bk"], np.float32)
    bv = np.asarray(inputs["bv"], np.float32)
    bo = np.asarray(inputs["bo"], np.float32)
    wgq = np.asarray(inputs["wgq"], np.float32)
    wgk = np.asarray(inputs["wgk"], np.float32)
    gb = float(np.asarray(inputs["gb"]))

    st, m01_b, midx = _prep_mask(mask)

    # x in [P, ND, S] batched-DMA layout
    xt_b = [[np.ascontiguousarray(
                x[b].T.reshape(ND, P, S).transpose(1, 0, 2)).astype(bf16)
             for b in range(B)] for x in (q, k, v)]

    def wslice(W, cols, scale=1.0):
        # W.T column slice [D, 128] -> [128, 8, 128] -> [128, 1024]
        wt = (W.T[:, cols:cols + CW] * scale).astype(bf16)
        return np.ascontiguousarray(
            wt.reshape(ND, P, CW).transpose(1, 0, 2).reshape(P, D))

    # q is pre-scaled by 0.5/sqrt(dk): scores arrive as s/2, and the
    # gate multiply computes (tanh(l/2)+1)*(s/2) = sigma(l)*s.
    qscale = 0.5 / np.sqrt(DK)
    ident = np.eye(P, dtype=bf16)

    meta = {
        "st": st, "midx": midx, "n_m01": m01_b[0].shape[1], "gb": gb,
        "use_bq": bool(np.any(bq)), "use_bk": bool(np.any(bk)),
        "use_bv": bool(np.any(bv)),
    }

    # gate weight row layouts: gq lands at rows h*DK of ug2 (compensated
    # for the q pre-scale); gkT lands at rows h*DK+1 of lk2.
    # ebias = -gb - ln2 for the u-exp; gb05 = gb/2 for the tanh bias
    ebias_h = np.full((P, 1), -gb - LN2, np.float32)
    gb05_h = np.full((P, 1), 0.5 * gb, np.float32)

    wgq_bd = np.zeros((P, DKP), np.float32)
    wgk_bd = np.zeros((P, HPC), np.float32)
    for h in range(HPC):
        wgq_bd[h * DK:(h + 1) * DK, h * DK] = wgq / qscale
        wgk_bd[h * DK:(h + 1) * DK, h] = wgk

    ngrp = NCORES // B          # head-groups per batch
    in_maps = []
    for c in range(NCORES):
        bc = c // ngrp          # batch of this core
        hg = c % ngrp           # head-group
        cols = [(hg * HPC + 0) * CW, (hg * HPC + 1) * CW]
        im = {
            "xqt": xt_b[0][bc], "xkt": xt_b[1][bc], "xvt": xt_b[2][bc],
            "wqt": np.ascontiguousarray(np.stack(
                [wslice(Wq, cl, qscale) for cl in cols]).transpose(1, 0, 2)),
            "wkt": np.ascontiguousarray(np.stack(
                [wslice(Wk, cl) for cl in cols]).transpose(1, 0, 2)),
            "wvt": np.ascontiguousarray(np.stack(
                [wslice(Wv, cl) for cl in cols]).transpose(1, 0, 2)),
            "wot": np.ascontiguousarray(np.stack(
                [Wo.T[cl:cl + CW, :].astype(bf16)
                 for cl in cols]).transpose(1, 0, 2)),
            "wgq": wgq_bd.astype(bf16), "wgk": wgk_bd.astype(bf16),
            "ebiasd": ebias_h, "gb05d": gb05_h,
            "identd": ident, "m01d": m01_b[bc],
            "onesd": np.ones((DK + 1, P), bf16),
            "bqt": np.ascontiguousarray(np.stack(
                [(bq[cl:cl + CW] * qscale).reshape(P, 1).astype(np.float32)
                 for cl in cols]).transpose(1, 0, 2)),
            "bkt": np.ascontiguousarray(np.stack(
                [bk[cl:cl + CW].reshape(P, 1).astype(np.float32)
                 for cl in cols]).transpose(1, 0, 2)),
            "bvt": np.ascontiguousarray(np.stack(
                [np.tile(bv[cl:cl + CW], (P, 1)).astype(np.float32)
                 for cl in cols]).transpose(1, 0, 2)),
        }
        in_maps.append(im)
    return meta, in_maps, bo


def kernel(**inputs):
    meta, in_maps, bo = _host_prep(inputs)

    key = (meta["st"].tobytes(), meta["gb"], meta["use_bq"],
           meta["use_bk"], meta["use_bv"], meta["n_m01"])
    if key not in _CACHE:
        _CACHE[key] = _build(meta)
    nc = _CACHE[key]

    from concourse.bass_utils import run_bass_kernel_spmd
    res = run_bass_kernel_spmd(
        nc, in_maps, core_ids=list(range(NCORES)),
        trace=bool(int(os.environ.get("KERNEL_TRACE", "0"))))
    out = np.zeros((B, S, D), np.float32)
    ngrp = NCORES // B
    for c, r in enumerate(res.results):
        out[c // ngrp] += r["outp"].astype(np.float32)
    out += bo
    if res.exec_time_ns is not None:
        print(f"HW exec time: {res.exec_time_ns} ns")
    return out


# revision 26
# speedup vs baseline: 1.0738x; 1.0091x over previous
"""Multi-head forgetting attention on 8 trn2 cores.

Sharding: 4 heads per core as 2 partition-slices ("hp") of 2 heads each
(head/tensor parallel). Each core receives the full (host-pre-transposed)
activations, its column slice of Wq/Wk/Wv, its row slice of Wo^T, and
produces a partial (S,D) bf16 output (both hp accumulated on-chip) which
the host sums in f32 (+ bo).

Gate structure: l = gq(q)+gk(k)+gb is materialised per 128x512 tile by a
2-contraction PE matmul from row-layout gq / gk vectors (bf16, PSUM), so
the ACT tanh pass needs no per-head bias and handles both heads per
instruction: sigma*s = (tanh(l/2)+1) * (s/2), with the 1/2 folded into
the host-side q scaling. For the aq=3 chunks the gate instead goes
through an exact reciprocal path (den = 0.5+0.5*u(q)v(k) on Pool from
exp vectors, reciprocal_approx_fast + multiply on DVE) which moves work
off the ACT engine. exp stays on ACT; the elementwise multiply reads
scores straight from PSUM (scores matmul emits bf16).

Other structure: flash-style streaming over k-tiles per 512-wide q chunk,
mixed 128x128 mask blocks multiplied by deduped 0/1 tiles (loaded once),
x tiles DMAed in 1MB batches, output projection accumulates both hp in
PSUM, one output DMA per 128-row slice.
"""

import os
import sys

sys.path.insert(0, "/opt/trn_rl_repo")

import numpy as np
import ml_dtypes

bf16 = ml_dtypes.bfloat16

B, S, D, H = 2, 2048, 1024, 16
DK = 64
NCORES = 8
HPC = 2          # head-pairs per core
CW = HPC * DK    # 128 per-slice head width
P = 128
QTW = 512        # q tile width (matmul free dim)
NQT = S // QTW   # 4
NKT = S // P     # 16 k tiles
NSL = S // P     # 16 q slices
ND = D // P      # 8 contraction tiles
DKP = DK + 2     # gq/gk row-layout tiles (2 rows per head-pair)
LN2 = 0.6931471805599453

# (aq, hp) chunks routed through the reciprocal gate path (off-ACT)
RECIP_PAIRS = {(3, 0), (3, 1)}

_CACHE = {}


def _prep_mask(mask):
    """Batch-union block table: 0 skip / 1 full / 2 mixed, plus per-batch
    0/1 tile contents for each union-mixed block (cores take their b's).
    Mixed tiles are deduped (consistently across batches) so each unique
    tile is loaded once."""
    m = np.asarray(mask).astype(bool)
    st = np.zeros((NKT, NSL), dtype=np.int8)
    uniq = {}
    m01 = [[] for _ in range(B)]
    midx = {}
    for i in range(NKT):
        for s in range(NSL):
            blks = [m[b, s * P:(s + 1) * P, i * P:(i + 1) * P]
                    for b in range(B)]
            alls = [blk.all() for blk in blks]
            anys = [blk.any() for blk in blks]
            if all(alls):
                st[i, s] = 1
            elif not any(anys):
                st[i, s] = 0
            else:
                st[i, s] = 2
                # used as matmul lhsT (applied transposed): keep blk
                # orientation so sc[k,q] += (blk[q,k]-1)*3e4
                tls = [np.ascontiguousarray(
                        (blks[b].astype(np.float32) - 1.0)
                        * 30000.0).astype(bf16) for b in range(B)]
                key = tuple(t.tobytes() for t in tls)
                if key not in uniq:
                    uniq[key] = len(m01[0])
                    for b in range(B):
                        m01[b].append(tls[b])
                midx[(i, s)] = uniq[key]
    if not m01[0]:
        for b in range(B):
            m01[b].append(np.zeros((P, P), dtype=bf16))
    # SBUF layout [P, n_uniq, P]
    return st, [np.stack(x, axis=1) for x in m01], midx


def _build(meta):
    """Build the (shared-across-cores) bass program."""
    import concourse.mybir as mybir
    import concourse.tile as tile
    from concourse import bacc

    st = meta["st"]
    midx = meta["midx"]
    n_m01 = meta["n_m01"]
    gb = meta["gb"]
    use_bq = meta["use_bq"]
    use_bk = meta["use_bk"]
    use_bv = meta["use_bv"]

    f32 = mybir.dt.float32
    b16 = mybir.dt.bfloat16
    Act = mybir.ActivationFunctionType
    Alu = mybir.AluOpType

    # block tables (batch-union)
    iv_qt = {qt: [i for i in range(NKT)
                  if any(st[i, 4 * qt + j] for j in range(4))]
             for qt in range(NQT)}
    valid_i = {s: [i for i in range(NKT) if st[i, s]]
               for s in range(NSL)}
    # attention chunk qt can only be emitted after projection chunk
    # mc[qt] (the latest chunk producing a k-tile it reads)
    mc = {qt: max((i // (QTW // P) for i in iv_qt[qt]), default=0)
          for qt in range(NQT)}

    nc = bacc.Bacc("TRN2", debug=False, enable_asserts=False,
                   num_devices=NCORES)

    xqt = nc.dram_tensor("xqt", (P, ND, S), b16, kind="ExternalInput")
    xkt = nc.dram_tensor("xkt", (P, ND, S), b16, kind="ExternalInput")
    xvt = nc.dram_tensor("xvt", (P, ND, S), b16, kind="ExternalInput")
    wqt = nc.dram_tensor("wqt", (P, HPC, D), b16, kind="ExternalInput")
    wkt = nc.dram_tensor("wkt", (P, HPC, D), b16, kind="ExternalInput")
    wvt = nc.dram_tensor("wvt", (P, HPC, D), b16, kind="ExternalInput")
    wot = nc.dram_tensor("wot", (P, HPC, D), b16, kind="ExternalInput")
    wgq = nc.dram_tensor("wgq", (P, DKP), b16, kind="ExternalInput")
    wgk = nc.dram_tensor("wgk", (P, HPC), b16, kind="ExternalInput")
    identd = nc.dram_tensor("identd", (P, P), b16, kind="ExternalInput")
    onesd = nc.dram_tensor("onesd", (DK + 1, P), b16, kind="ExternalInput")
    m01d = nc.dram_tensor("m01d", (P, n_m01, P), b16, kind="ExternalInput")
    ebiasd = nc.dram_tensor("ebiasd", (P, 1), f32, kind="ExternalInput")
    gb05d = nc.dram_tensor("gb05d", (P, 1), f32, kind="ExternalInput")
    bqt = nc.dram_tensor("bqt", (P, HPC, 1), f32, kind="ExternalInput")
    bkt = nc.dram_tensor("bkt", (P, HPC, 1), f32, kind="ExternalInput")
    bvt = nc.dram_tensor("bvt", (P, HPC, P), f32, kind="ExternalInput")
    outp = nc.dram_tensor("outp", (S, D), b16, kind="ExternalOutput")

    with tile.TileContext(nc) as tc:
        from contextlib import ExitStack
        with ExitStack() as ctx:
            cst = ctx.enter_context(tc.tile_pool(name="cst", bufs=1))
            per = ctx.enter_context(tc.tile_pool(name="per", bufs=1))
            strm = ctx.enter_context(tc.tile_pool(name="strm", bufs=1))
            work = ctx.enter_context(tc.tile_pool(name="work", bufs=2))
            prb = ctx.enter_context(tc.tile_pool(name="prb", bufs=1))
            mis = ctx.enter_context(
                tc.tile_pool(name="mis", bufs=2, space="PSUM"))
            scp = ctx.enter_context(
                tc.tile_pool(name="scp", bufs=4, space="PSUM"))
            att = ctx.enter_context(
                tc.tile_pool(name="att", bufs=2, space="PSUM"))

            # ---- constants ----
            wq_sb = cst.tile([P, HPC, D], b16, name="wq_sb")
            wk_sb = cst.tile([P, HPC, D], b16, name="wk_sb")
            wv_sb = cst.tile([P, HPC, D], b16, name="wv_sb")
            wo_sb = cst.tile([P, HPC, D], b16, name="wo_sb")
            wgq_sb = cst.tile([P, DKP], b16, name="wgq_sb")
            wgk_sb = cst.tile([P, HPC], b16, name="wgk_sb")
            id_sb = cst.tile([P, P], b16, name="id_sb")
            ones_sb = cst.tile([DK + 1, P], b16, name="ones_sb")
            m01_sb = cst.tile([P, n_m01, P], b16, name="m01_sb")
            ebias = cst.tile([P, 1], f32, name="ebias")
            gb05 = cst.tile([P, 1], f32, name="gb05")
            bq_sb = cst.tile([P, HPC, 1], f32, name="bq_sb")
            bk_sb = cst.tile([P, HPC, 1], f32, name="bk_sb")
            bv_sb = cst.tile([P, HPC, P], f32, name="bv_sb")
            nc.sync.dma_start(wq_sb[:], wqt[:, :, :])
            nc.sync.dma_start(wk_sb[:], wkt[:, :, :])
            nc.sync.dma_start(wv_sb[:], wvt[:, :, :])
            nc.sync.dma_start(wo_sb[:], wot[:, :, :])
            nc.sync.dma_start(wgq_sb[:], wgq[:, :])
            nc.sync.dma_start(wgk_sb[:], wgk[:, :])
            nc.sync.dma_start(id_sb[:], identd[:, :])
            nc.sync.dma_start(ones_sb[:], onesd[:, :])
            nc.scalar.dma_start(m01_sb[:], m01d[:, :, :])
            nc.sync.dma_start(ebias[:], ebiasd[:, :])
            nc.sync.dma_start(gb05[:], gb05d[:, :])
            if use_bq:
                nc.sync.dma_start(bq_sb[:], bqt[:, :, :])
            if use_bk:
                nc.sync.dma_start(bk_sb[:], bkt[:, :, :])
            if use_bv:
                nc.sync.dma_start(bv_sb[:], bvt[:, :, :])

            # per-hp persistent buffers
            qt_sb = [per.tile([P, S], b16, name=f"qt{hp}_sb", tag=f"qt{hp}")
                     for hp in range(HPC)]
            kt_sb = [per.tile([P, S], b16, name=f"kt{hp}_sb", tag=f"kt{hp}")
                     for hp in range(HPC)]
            v2_sb = [per.tile([P, NKT, HPC, DK + 1], b16,
                              name=f"v{hp}_sb", tag=f"v{hp}")
                     for hp in range(HPC)]
            ug2_sb = [per.tile([DKP, S], b16, name=f"ug{hp}_sb",
                               tag=f"ug{hp}") for hp in range(HPC)]
            gqb_sb = [per.tile([P, HPC, S], b16, name=f"gqb{hp}_sb",
                               tag=f"gqb{hp}") for hp in range(HPC)]
            ubc_sb = [per.tile([P, HPC, S], b16, name=f"ubc{hp}_sb",
                               tag=f"ubc{hp}") for hp in range(HPC)]
            gk05_sb = [per.tile([P, HPC, NKT], f32, name=f"gk05{hp}_sb",
                                tag=f"gk05{hp}") for hp in range(HPC)]
            vex_sb = [per.tile([P, HPC, NKT], f32, name=f"vex{hp}_sb",
                               tag=f"vex{hp}") for hp in range(HPC)]

            for hp in range(HPC):
                nc.vector.memset(v2_sb[hp][:, :, :, DK], 1.0)

            for qt in range(NQT):
                # ===== projection chunk qt (x tiles shared by both hp) ====
                q0 = qt * QTW
                # Q / K chunk qt: one batched x tile feeds both hp matmuls
                for (xsrc, wsb, osb, bias_sb, use_b, dmae, xtag) in (
                        (xqt, wq_sb, qt_sb, bq_sb, use_bq, nc.scalar, "xq"),
                        (xkt, wk_sb, kt_sb, bk_sb, use_bk, nc.sync, "xk")):
                    xt = strm.tile([P, ND, QTW], b16, tag=xtag, name="xt")
                    dmae.dma_start(xt[:], xsrc[:, :, q0:q0 + QTW])
                    pps = [scp.tile([P, QTW], f32, tag="sc",
                                    name=f"pps{hp}") for hp in range(HPC)]
                    for dt in range(ND):
                        for hp in range(HPC):
                            nc.tensor.matmul(
                                pps[hp][:],
                                lhsT=wsb[:, hp, dt * P:(dt + 1) * P],
                                rhs=xt[:, dt, :],
                                start=(dt == 0), stop=(dt == ND - 1))
                    for hp in range(HPC):
                        dst = osb[hp][:, q0:q0 + QTW]
                        if use_b:
                            nc.scalar.activation(
                                dst, pps[hp][:], Act.Identity,
                                bias=bias_sb[:, hp, :])
                        elif hp:
                            nc.scalar.copy(dst, pps[hp][:])
                        else:
                            nc.vector.tensor_copy(dst, pps[hp][:])

                # V slices 4qt..4qt+3
                xv = strm.tile([P, ND, QTW], b16, tag="xv", name="xv")
                nc.sync.dma_start(xv[:], xvt[:, :, q0:q0 + QTW])
                for hp in range(HPC):
                    for sj in range(QTW // P):
                        sl = qt * (QTW // P) + sj
                        vps = scp.tile([P, HPC, DK], f32, tag="sc",
                                       name="vps")
                        for dt in range(ND):
                            nc.tensor.matmul(
                                vps[:],
                                lhsT=xv[:, dt, sj * P:(sj + 1) * P],
                                rhs=wv_sb[:, hp, dt * P:(dt + 1) * P],
                                start=(dt == 0), stop=(dt == ND - 1))
                        if use_bv:
                            for h in range(HPC):
                                nc.vector.tensor_add(
                                    vps[:, h, :], vps[:, h, :],
                                    bv_sb[:, hp, h * DK:(h + 1) * DK])
                        # both heads in one strided copy
                        if sj % 2:
                            nc.scalar.copy(
                                v2_sb[hp][:, sl, :, 0:DK], vps[:])
                        else:
                            nc.vector.tensor_copy(
                                v2_sb[hp][:, sl, :, 0:DK], vps[:])

                for hp in range(HPC):
                    # gq row chunk, then its partition-broadcast tiles
                    gps = mis.tile([DKP, QTW], f32, tag="mis", name="gps")
                    nc.tensor.matmul(
                        gps[:], lhsT=wgq_sb[:],
                        rhs=qt_sb[hp][:, q0:q0 + QTW],
                        start=True, stop=True)
                    nc.scalar.copy(ug2_sb[hp][:, q0:q0 + QTW], gps[:])
                    for h in range(HPC):
                        gqp = mis.tile([P, QTW], f32, tag="mis",
                                       name="gqp")
                        nc.tensor.matmul(
                            gqp[:],
                            lhsT=ones_sb[h * DK:h * DK + 1, :],
                            rhs=ug2_sb[hp][h * DK:h * DK + 1,
                                           q0:q0 + QTW],
                            start=True, stop=True)
                        if h:
                            nc.scalar.copy(
                                gqb_sb[hp][:, h, q0:q0 + QTW], gqp[:])
                        else:
                            nc.vector.tensor_copy(
                                gqb_sb[hp][:, h, q0:q0 + QTW], gqp[:])
                        nc.scalar.activation(
                            ubc_sb[hp][:, h, q0:q0 + QTW], gqp[:],
                            Act.Exp, scale=-1.0, bias=ebias[:, :])

                    # per-partition gk: gk05 (tanh bias) + vex (recip)
                    for h in range(HPC):
                        hsl = slice(h * DK, (h + 1) * DK)
                        gkp = mis.tile([P, QTW // P], f32, tag="mis",
                                       name="gkp")
                        for j in range(QTW // P):
                            i = qt * (QTW // P) + j
                            nc.tensor.matmul(
                                gkp[:, j:j + 1],
                                lhsT=kt_sb[hp][hsl, i * P:(i + 1) * P],
                                rhs=wgk_sb[hsl, h:h + 1],
                                start=(j == 0), stop=(j == QTW // P - 1),
                                skip_group_check=True)
                        nc.scalar.activation(
                            gk05_sb[hp][:, h, qt * (QTW // P):
                                        (qt + 1) * (QTW // P)],
                            gkp[:], Act.Identity, scale=0.5,
                            bias=gb05[:, :])
                        nc.scalar.activation(
                            vex_sb[hp][:, h, qt * (QTW // P):
                                       (qt + 1) * (QTW // P)],
                            gkp[:], Act.Exp, scale=-1.0)

                # ===== attention chunks whose k-tiles are now ready ====
                ready = [aq for aq in range(NQT)
                         if mc[aq] == qt or (qt == NQT - 1 and mc[aq] > qt)]
                for aq in ready:
                  a0 = aq * QTW
                  otts = {}
                  for hp in range(HPC):
                      probs = {}
                      tiles = []
                      for n_i, i in enumerate(iv_qt[aq]):
                          recip = (n_i % 3 == 1)
                          # first non-skip 128-slice of this (i, aq)
                          sjlo = min(j for j in range(QTW // P)
                                     if st[i, aq * (QTW // P) + j])
                          tiles.append((i, recip, sjlo * P))
                      for (i, recip, off) in tiles:
                          pre = {}
                          for h in range(HPC):
                              if recip:
                                  dn = work.tile([P, QTW], f32,
                                                 tag="den", name="den",
                                                 bufs=4)
                                  nc.gpsimd.tensor_scalar(
                                      dn[:, off:],
                                      ubc_sb[hp][:, h, a0 + off:a0 + QTW],
                                      vex_sb[hp][:, h, i:i + 1], 0.5,
                                      Alu.mult, Alu.add)
                                  pre[(i, h)] = dn
                              else:
                                  tnh = work.tile([P, QTW], b16,
                                                  tag="tnh", name="tnh",
                                                  bufs=6)
                                  nc.scalar.activation(
                                      tnh[:, off:],
                                      gqb_sb[hp][:, h, a0 + off:a0 + QTW],
                                      Act.Tanh, scale=0.5,
                                      bias=gk05_sb[hp][:, h, i:i + 1])
                                  pre[(i, h)] = tnh
                          p3 = prb.tile([P, HPC, QTW], b16, tag=f"pr{i}",
                                        name=f"pr{i}",
                                        bufs=2 if i < 8 else 1)
                          g3 = work.tile([P, HPC, QTW], b16, tag="gat",
                                         name="gat")
                          sjlo = off // P
                          mixed = [sj for sj in range(sjlo, QTW // P)
                                   if st[i, aq * (QTW // P) + sj] == 2]
                          for h in range(HPC):
                              hsl = slice(h * DK, (h + 1) * DK)
                              sch = scp.tile([P, QTW], f32, tag="sc",
                                             name="sch")
                              nc.tensor.matmul(
                                  sch[:, off:],
                                  lhsT=kt_sb[hp][hsl, i * P:(i + 1) * P],
                                  rhs=qt_sb[hp][hsl, a0 + off:a0 + QTW],
                                  start=True, stop=not mixed)
                              # additive mask bias: sc += m01^T . I
                              for n, sj in enumerate(mixed):
                                  s = aq * (QTW // P) + sj
                                  nc.tensor.matmul(
                                      sch[:, sj * P:(sj + 1) * P],
                                      lhsT=m01_sb[:, midx[(i, s)], :],
                                      rhs=id_sb[:],
                                      start=False,
                                      stop=(n == len(mixed) - 1),
                                      skip_group_check=True)
                              if recip:
                                  rc = work.tile([P, QTW], f32, tag="rec",
                                                 name="rec", bufs=3)
                                  nc.vector.reciprocal_approx_fast(
                                      rc[:, off:], pre[(i, h)][:, off:])
                                  nc.vector.tensor_tensor(
                                      g3[:, h, off:], sch[:, off:],
                                      rc[:, off:], Alu.mult)
                              else:
                                  nc.vector.scalar_tensor_tensor(
                                      g3[:, h, off:], pre[(i, h)][:, off:],
                                      1.0, sch[:, off:],
                                      Alu.add, Alu.mult)
                          nc.scalar.activation(
                              p3[:, :, off:], g3[:, :, off:], Act.Exp)
                          probs[i] = p3

                      # attn @ V, normalize, transpose; out-proj deferred
                      ott = work.tile([P, QTW // P, P], b16,
                                      tag=f"ott{hp}", name=f"ott{hp}")
                      otts[hp] = ott
                      for sj in range(QTW // P):
                          s = aq * (QTW // P) + sj
                          onat = work.tile([P, P], b16, tag="onat",
                                           name="onat")
                          ops = att.tile([P, HPC * (DK + 1)], f32, tag="o",
                                         name="ops")
                          vi = valid_i[s]
                          for h in range(HPC):
                              ob = h * (DK + 1)
                              if not vi:
                                  nc.vector.memset(
                                      ops[:, ob:ob + DK + 1], 0.0)
                              for n, i in enumerate(vi):
                                  nc.tensor.matmul(
                                      ops[:, ob:ob + DK + 1],
                                      lhsT=probs[i][:, h,
                                                    sj * P:(sj + 1) * P],
                                      rhs=v2_sb[hp][:, i, h, :],
                                      start=(n == 0),
                                      stop=(n == len(vi) - 1),
                                      skip_group_check=True)
                              recv = work.tile([P, 1], f32, tag="recip",
                                               name="recip", bufs=4)
                              nc.vector.reciprocal_approx_fast(
                                  recv[:], ops[:, ob + DK:ob + DK + 1])
                              nc.vector.tensor_scalar_mul(
                                  onat[:, h * DK:(h + 1) * DK],
                                  ops[:, ob:ob + DK], recv[:])
                          trp = mis.tile([P, P], b16, tag="mis", name="trp")
                          nc.tensor.transpose(trp[:], onat[:], id_sb[:])
                          nc.vector.tensor_copy(ott[:, sj, :], trp[:])
                  # output projection: accumulate both hp per s-slice
                  for sj in range(QTW // P):
                      s = aq * (QTW // P) + sj
                      po = work.tile([P, 2, QTW], b16, tag="po",
                                     name="po", bufs=3)
                      for nt in range(2):
                          pps2 = mis.tile([P, QTW], f32, tag="mis",
                                          name="fps")
                          for hp in range(HPC):
                              nc.tensor.matmul(
                                  pps2[:],
                                  lhsT=otts[hp][:, sj, :],
                                  rhs=wo_sb[:, hp, nt * QTW:(nt + 1) * QTW],
                                  start=(hp == 0), stop=(hp == HPC - 1))
                          if nt:
                              nc.scalar.copy(po[:, nt, :], pps2[:])
                          else:
                              nc.vector.tensor_copy(po[:, nt, :], pps2[:])
                      nc.sync.dma_start(
                          outp[s * P:(s + 1) * P, :], po[:])
    nc.compile()
    return nc


def _host_prep(inputs):
    q = np.asarray(inputs["query"], np.float32)
    k = np.asarray(inputs["key"], np.float32)
    v = np.asarray(inputs["value"], np.float32)
    mask = np.asarray(inputs["mask"])
    Wq = np.asarray(inputs["Wq"], np.float32)
    Wk = np.asarray(inputs["Wk"], np.float32)
    Wv = np.asarray(inputs["Wv"], np.float32)
    Wo = np.asarray(inputs["Wo"], np.float32)
    bq = np.asarray(inputs["bq"], np.float32)
    bk = np.asarray(inputs["bk"], np.float32)
    bv = np.asarray(inputs["bv"], np.float32)
    bo = np.asarray(inputs["bo"], np.float32)
    wgq = np.asarray(inputs["wgq"], np.float32)
    wgk = np.asarray(inputs["wgk"], np.float32)
    gb = float(np.asarray(inputs["gb"]))

    st, m01_b, midx = _prep_mask(mask)

    # x in [P, ND, S] batched-DMA layout
    xt_b = [[np.ascontiguousarray(
                x[b].T.reshape(ND, P, S).transpose(1, 0, 2)).astype(bf16)
             for b in range(B)] for x in (q, k, v)]

    def wslice(W, cols, scale=1.0):
        # W.T column slice [D, 128] -> [128, 8, 128] -> [128, 1024]
        wt = (W.T[:, cols:cols + CW] * scale).astype(bf16)
        return np.ascontiguousarray(
            wt.reshape(ND, P, CW).transpose(1, 0, 2).reshape(P, D))

    # q is pre-scaled by 0.5/sqrt(dk): scores arrive as s/2, and the
    # gate multiply computes (tanh(l/2)+1)*(s/2) = sigma(l)*s.
    qscale = 0.5 / np.sqrt(DK)
    ident = np.eye(P, dtype=bf16)

    meta = {
        "st": st, "midx": midx, "n_m01": m01_b[0].shape[1], "gb": gb,
        "use_bq": bool(np.any(bq)), "use_bk": bool(np.any(bk)),
        "use_bv": bool(np.any(bv)),
    }

    # gate weight row layouts: gq lands at rows h*DK of ug2 (compensated
    # for the q pre-scale); gkT lands at rows h*DK+1 of lk2.
    # ebias = -gb - ln2 for the u-exp; gb05 = gb/2 for the tanh bias
    ebias_h = np.full((P, 1), -gb - LN2, np.float32)
    gb05_h = np.full((P, 1), 0.5 * gb, np.float32)

    wgq_bd = np.zeros((P, DKP), np.float32)
    wgk_bd = np.zeros((P, HPC), np.float32)
    for h in range(HPC):
        wgq_bd[h * DK:(h + 1) * DK, h * DK] = wgq / qscale
        wgk_bd[h * DK:(h + 1) * DK, h] = wgk

    ngrp = NCORES // B          # head-groups per batch
    in_maps = []
    for c in range(NCORES):
        bc = c // ngrp          # batch of this core
        hg = c % ngrp           # head-group
        cols = [(hg * HPC + 0) * CW, (hg * HPC + 1) * CW]
        im = {
            "xqt": xt_b[0][bc], "xkt": xt_b[1][bc], "xvt": xt_b[2][bc],
            "wqt": np.ascontiguousarray(np.stack(
                [wslice(Wq, cl, qscale) for cl in cols]).transpose(1, 0, 2)),
            "wkt": np.ascontiguousarray(np.stack(
                [wslice(Wk, cl) for cl in cols]).transpose(1, 0, 2)),
            "wvt": np.ascontiguousarray(np.stack(
                [wslice(Wv, cl) for cl in cols]).transpose(1, 0, 2)),
            "wot": np.ascontiguousarray(np.stack(
                [Wo.T[cl:cl + CW, :].astype(bf16)
                 for cl in cols]).transpose(1, 0, 2)),
            "wgq": wgq_bd.astype(bf16), "wgk": wgk_bd.astype(bf16),
            "ebiasd": ebias_h, "gb05d": gb05_h,
            "identd": ident, "m01d": m01_b[bc],
            "onesd": np.ones((DK + 1, P), bf16),
            "bqt": np.ascontiguousarray(np.stack(
                [(bq[cl:cl + CW] * qscale).reshape(P, 1).astype(np.float32)
                 for cl in cols]).transpose(1, 0, 2)),
            "bkt": np.ascontiguousarray(np.stack(
                [bk[cl:cl + CW].reshape(P, 1).astype(np.float32)
                 for cl in cols]).transpose(1, 0, 2)),
            "bvt": np.ascontiguousarray(np.stack(
                [np.tile(bv[cl:cl + CW], (P, 1)).astype(np.float32)
                 for cl in cols]).transpose(1, 0, 2)),
        }
        in_maps.append(im)
    return meta, in_maps, bo


def kernel(**inputs):
    meta, in_maps, bo = _host_prep(inputs)

    key = (meta["st"].tobytes(), meta["gb"], meta["use_bq"],
           meta["use_bk"], meta["use_bv"], meta["n_m01"])
    if key not in _CACHE:
        _CACHE[key] = _build(meta)
    nc = _CACHE[key]

    from concourse.bass_utils import run_bass_kernel_spmd
    res = run_bass_kernel_spmd(
        nc, in_maps, core_ids=list(range(NCORES)),
        trace=bool(int(os.environ.get("KERNEL_TRACE", "0"))))
    out = np.zeros((B, S, D), np.float32)
    ngrp = NCORES // B
    for c, r in enumerate(res.results):
        out[c // ngrp] += r["outp"].astype(np.float32)
    out += bo
    if res.exec_time_ns is not None:
        print(f"HW exec time: {res.exec_time_ns} ns")
    return out


# revision 27
# speedup vs baseline: 1.0812x; 1.0068x over previous
"""Multi-head forgetting attention on 8 trn2 cores.

Sharding: 4 heads per core as 2 partition-slices ("hp") of 2 heads each
(head/tensor parallel). Each core receives the full (host-pre-transposed)
activations, its column slice of Wq/Wk/Wv, its row slice of Wo^T, and
produces a partial (S,D) bf16 output (both hp accumulated on-chip) which
the host sums in f32 (+ bo).

Gate structure: l = gq(q)+gk(k)+gb is materialised per 128x512 tile by a
2-contraction PE matmul from row-layout gq / gk vectors (bf16, PSUM), so
the ACT tanh pass needs no per-head bias and handles both heads per
instruction: sigma*s = (tanh(l/2)+1) * (s/2), with the 1/2 folded into
the host-side q scaling. For the aq=3 chunks the gate instead goes
through an exact reciprocal path (den = 0.5+0.5*u(q)v(k) on Pool from
exp vectors, reciprocal_approx_fast + multiply on DVE) which moves work
off the ACT engine. exp stays on ACT; the elementwise multiply reads
scores straight from PSUM (scores matmul emits bf16).

Other structure: flash-style streaming over k-tiles per 512-wide q chunk,
mixed 128x128 mask blocks multiplied by deduped 0/1 tiles (loaded once),
x tiles DMAed in 1MB batches, output projection accumulates both hp in
PSUM, one output DMA per 128-row slice.
"""

import os
import sys

sys.path.insert(0, "/opt/trn_rl_repo")

import numpy as np
import ml_dtypes

bf16 = ml_dtypes.bfloat16

B, S, D, H = 2, 2048, 1024, 16
DK = 64
NCORES = 8
HPC = 2          # head-pairs per core
CW = HPC * DK    # 128 per-slice head width
P = 128
QTW = 512        # q tile width (matmul free dim)
NQT = S // QTW   # 4
NKT = S // P     # 16 k tiles
NSL = S // P     # 16 q slices
ND = D // P      # 8 contraction tiles
DKP = DK + 2     # gq/gk row-layout tiles (2 rows per head-pair)
LN2 = 0.6931471805599453

# (aq, hp) chunks routed through the reciprocal gate path (off-ACT)
RECIP_PAIRS = {(3, 0), (3, 1)}

_CACHE = {}


def _prep_mask(mask):
    """Batch-union block table: 0 skip / 1 full / 2 mixed, plus per-batch
    0/1 tile contents for each union-mixed block (cores take their b's).
    Mixed tiles are deduped (consistently across batches) so each unique
    tile is loaded once."""
    m = np.asarray(mask).astype(bool)
    st = np.zeros((NKT, NSL), dtype=np.int8)
    uniq = {}
    m01 = [[] for _ in range(B)]
    midx = {}
    for i in range(NKT):
        for s in range(NSL):
            blks = [m[b, s * P:(s + 1) * P, i * P:(i + 1) * P]
                    for b in range(B)]
            alls = [blk.all() for blk in blks]
            anys = [blk.any() for blk in blks]
            if all(alls):
                st[i, s] = 1
            elif not any(anys):
                st[i, s] = 0
            else:
                st[i, s] = 2
                # used as matmul lhsT (applied transposed): keep blk
                # orientation so sc[k,q] += (blk[q,k]-1)*3e4
                tls = [np.ascontiguousarray(
                        (blks[b].astype(np.float32) - 1.0)
                        * 30000.0).astype(bf16) for b in range(B)]
                key = tuple(t.tobytes() for t in tls)
                if key not in uniq:
                    uniq[key] = len(m01[0])
                    for b in range(B):
                        m01[b].append(tls[b])
                midx[(i, s)] = uniq[key]
    if not m01[0]:
        for b in range(B):
            m01[b].append(np.zeros((P, P), dtype=bf16))
    # SBUF layout [P, n_uniq, P]
    return st, [np.stack(x, axis=1) for x in m01], midx


def _build(meta):
    """Build the (shared-across-cores) bass program."""
    import concourse.mybir as mybir
    import concourse.tile as tile
    from concourse import bacc

    st = meta["st"]
    midx = meta["midx"]
    n_m01 = meta["n_m01"]
    gb = meta["gb"]
    use_bq = meta["use_bq"]
    use_bk = meta["use_bk"]
    use_bv = meta["use_bv"]

    f32 = mybir.dt.float32
    b16 = mybir.dt.bfloat16
    Act = mybir.ActivationFunctionType
    Alu = mybir.AluOpType

    # block tables (batch-union)
    iv_qt = {qt: [i for i in range(NKT)
                  if any(st[i, 4 * qt + j] for j in range(4))]
             for qt in range(NQT)}
    valid_i = {s: [i for i in range(NKT) if st[i, s]]
               for s in range(NSL)}
    # attention chunk qt can only be emitted after projection chunk
    # mc[qt] (the latest chunk producing a k-tile it reads)
    mc = {qt: max((i // (QTW // P) for i in iv_qt[qt]), default=0)
          for qt in range(NQT)}

    nc = bacc.Bacc("TRN2", debug=False, enable_asserts=False,
                   num_devices=NCORES)

    xqt = nc.dram_tensor("xqt", (P, ND, S), b16, kind="ExternalInput")
    xkt = nc.dram_tensor("xkt", (P, ND, S), b16, kind="ExternalInput")
    xvt = nc.dram_tensor("xvt", (P, ND, S), b16, kind="ExternalInput")
    wqt = nc.dram_tensor("wqt", (P, HPC, D), b16, kind="ExternalInput")
    wkt = nc.dram_tensor("wkt", (P, HPC, D), b16, kind="ExternalInput")
    wvt = nc.dram_tensor("wvt", (P, HPC, D), b16, kind="ExternalInput")
    wot = nc.dram_tensor("wot", (P, HPC, D), b16, kind="ExternalInput")
    wgq = nc.dram_tensor("wgq", (P, DKP), b16, kind="ExternalInput")
    wgk = nc.dram_tensor("wgk", (P, HPC), b16, kind="ExternalInput")
    identd = nc.dram_tensor("identd", (P, P), b16, kind="ExternalInput")
    onesd = nc.dram_tensor("onesd", (DK + 1, P), b16, kind="ExternalInput")
    m01d = nc.dram_tensor("m01d", (P, n_m01, P), b16, kind="ExternalInput")
    ebiasd = nc.dram_tensor("ebiasd", (P, 1), f32, kind="ExternalInput")
    gb05d = nc.dram_tensor("gb05d", (P, 1), f32, kind="ExternalInput")
    bqt = nc.dram_tensor("bqt", (P, HPC, 1), f32, kind="ExternalInput")
    bkt = nc.dram_tensor("bkt", (P, HPC, 1), f32, kind="ExternalInput")
    bvt = nc.dram_tensor("bvt", (P, HPC, P), f32, kind="ExternalInput")
    outp = nc.dram_tensor("outp", (S, D), b16, kind="ExternalOutput")

    with tile.TileContext(nc) as tc:
        from contextlib import ExitStack
        with ExitStack() as ctx:
            cst = ctx.enter_context(tc.tile_pool(name="cst", bufs=1))
            per = ctx.enter_context(tc.tile_pool(name="per", bufs=1))
            strm = ctx.enter_context(tc.tile_pool(name="strm", bufs=1))
            work = ctx.enter_context(tc.tile_pool(name="work", bufs=2))
            prb = ctx.enter_context(tc.tile_pool(name="prb", bufs=1))
            mis = ctx.enter_context(
                tc.tile_pool(name="mis", bufs=2, space="PSUM"))
            scp = ctx.enter_context(
                tc.tile_pool(name="scp", bufs=4, space="PSUM"))
            att = ctx.enter_context(
                tc.tile_pool(name="att", bufs=2, space="PSUM"))

            # ---- constants ----
            wq_sb = cst.tile([P, HPC, D], b16, name="wq_sb")
            wk_sb = cst.tile([P, HPC, D], b16, name="wk_sb")
            wv_sb = cst.tile([P, HPC, D], b16, name="wv_sb")
            wo_sb = cst.tile([P, HPC, D], b16, name="wo_sb")
            wgq_sb = cst.tile([P, DKP], b16, name="wgq_sb")
            wgk_sb = cst.tile([P, HPC], b16, name="wgk_sb")
            id_sb = cst.tile([P, P], b16, name="id_sb")
            ones_sb = cst.tile([DK + 1, P], b16, name="ones_sb")
            m01_sb = cst.tile([P, n_m01, P], b16, name="m01_sb")
            ebias = cst.tile([P, 1], f32, name="ebias")
            gb05 = cst.tile([P, 1], f32, name="gb05")
            bq_sb = cst.tile([P, HPC, 1], f32, name="bq_sb")
            bk_sb = cst.tile([P, HPC, 1], f32, name="bk_sb")
            bv_sb = cst.tile([P, HPC, P], f32, name="bv_sb")
            nc.sync.dma_start(wq_sb[:], wqt[:, :, :])
            nc.sync.dma_start(wk_sb[:], wkt[:, :, :])
            nc.sync.dma_start(wv_sb[:], wvt[:, :, :])
            nc.sync.dma_start(wo_sb[:], wot[:, :, :])
            nc.sync.dma_start(wgq_sb[:], wgq[:, :])
            nc.sync.dma_start(wgk_sb[:], wgk[:, :])
            nc.sync.dma_start(id_sb[:], identd[:, :])
            nc.sync.dma_start(ones_sb[:], onesd[:, :])
            nc.scalar.dma_start(m01_sb[:], m01d[:, :, :])
            nc.sync.dma_start(ebias[:], ebiasd[:, :])
            nc.sync.dma_start(gb05[:], gb05d[:, :])
            if use_bq:
                nc.sync.dma_start(bq_sb[:], bqt[:, :, :])
            if use_bk:
                nc.sync.dma_start(bk_sb[:], bkt[:, :, :])
            if use_bv:
                nc.sync.dma_start(bv_sb[:], bvt[:, :, :])

            # per-hp persistent buffers
            qt_sb = [per.tile([P, S], b16, name=f"qt{hp}_sb", tag=f"qt{hp}")
                     for hp in range(HPC)]
            kt_sb = [per.tile([P, S], b16, name=f"kt{hp}_sb", tag=f"kt{hp}")
                     for hp in range(HPC)]
            v2_sb = [per.tile([P, NKT, HPC, DK + 1], b16,
                              name=f"v{hp}_sb", tag=f"v{hp}")
                     for hp in range(HPC)]
            ug2_sb = [per.tile([DKP, S], b16, name=f"ug{hp}_sb",
                               tag=f"ug{hp}") for hp in range(HPC)]
            gqb_sb = [per.tile([P, HPC, S], b16, name=f"gqb{hp}_sb",
                               tag=f"gqb{hp}") for hp in range(HPC)]
            ubc_sb = [per.tile([P, HPC, S], b16, name=f"ubc{hp}_sb",
                               tag=f"ubc{hp}") for hp in range(HPC)]
            gk05_sb = [per.tile([P, HPC, NKT], f32, name=f"gk05{hp}_sb",
                                tag=f"gk05{hp}") for hp in range(HPC)]
            vex_sb = [per.tile([P, HPC, NKT], f32, name=f"vex{hp}_sb",
                               tag=f"vex{hp}") for hp in range(HPC)]

            for hp in range(HPC):
                nc.vector.memset(v2_sb[hp][:, :, :, DK], 1.0)

            for qt in range(NQT):
                # ===== projection chunk qt (x tiles shared by both hp) ====
                q0 = qt * QTW
                # Q / K chunk qt: one batched x tile feeds both hp matmuls
                for (xsrc, wsb, osb, bias_sb, use_b, dmae, xtag) in (
                        (xqt, wq_sb, qt_sb, bq_sb, use_bq, nc.scalar, "xq"),
                        (xkt, wk_sb, kt_sb, bk_sb, use_bk, nc.sync, "xk")):
                    xt = strm.tile([P, ND, QTW], b16, tag=xtag, name="xt")
                    dmae.dma_start(xt[:], xsrc[:, :, q0:q0 + QTW])
                    pps = [scp.tile([P, QTW], f32, tag="sc",
                                    name=f"pps{hp}") for hp in range(HPC)]
                    for dt in range(ND):
                        for hp in range(HPC):
                            nc.tensor.matmul(
                                pps[hp][:],
                                lhsT=wsb[:, hp, dt * P:(dt + 1) * P],
                                rhs=xt[:, dt, :],
                                start=(dt == 0), stop=(dt == ND - 1))
                    for hp in range(HPC):
                        dst = osb[hp][:, q0:q0 + QTW]
                        if use_b:
                            nc.scalar.activation(
                                dst, pps[hp][:], Act.Identity,
                                bias=bias_sb[:, hp, :])
                        elif hp:
                            nc.scalar.copy(dst, pps[hp][:])
                        else:
                            nc.vector.tensor_copy(dst, pps[hp][:])

                # V slices 4qt..4qt+3
                xv = strm.tile([P, ND, QTW], b16, tag="xv", name="xv")
                nc.sync.dma_start(xv[:], xvt[:, :, q0:q0 + QTW])
                for hp in range(HPC):
                    for sj in range(QTW // P):
                        sl = qt * (QTW // P) + sj
                        vps = scp.tile([P, HPC, DK], f32, tag="sc",
                                       name="vps")
                        for dt in range(ND):
                            nc.tensor.matmul(
                                vps[:],
                                lhsT=xv[:, dt, sj * P:(sj + 1) * P],
                                rhs=wv_sb[:, hp, dt * P:(dt + 1) * P],
                                start=(dt == 0), stop=(dt == ND - 1))
                        if use_bv:
                            for h in range(HPC):
                                nc.vector.tensor_add(
                                    vps[:, h, :], vps[:, h, :],
                                    bv_sb[:, hp, h * DK:(h + 1) * DK])
                        # both heads in one strided copy
                        if sj % 2:
                            nc.scalar.copy(
                                v2_sb[hp][:, sl, :, 0:DK], vps[:])
                        else:
                            nc.vector.tensor_copy(
                                v2_sb[hp][:, sl, :, 0:DK], vps[:])

                for hp in range(HPC):
                    # gq row chunk, then its partition-broadcast tiles
                    gps = mis.tile([DKP, QTW], f32, tag="mis", name="gps")
                    nc.tensor.matmul(
                        gps[:], lhsT=wgq_sb[:],
                        rhs=qt_sb[hp][:, q0:q0 + QTW],
                        start=True, stop=True)
                    nc.scalar.copy(ug2_sb[hp][:, q0:q0 + QTW], gps[:])
                    for h in range(HPC):
                        gqp = mis.tile([P, QTW], f32, tag="mis",
                                       name="gqp")
                        nc.tensor.matmul(
                            gqp[:],
                            lhsT=ones_sb[h * DK:h * DK + 1, :],
                            rhs=ug2_sb[hp][h * DK:h * DK + 1,
                                           q0:q0 + QTW],
                            start=True, stop=True)
                        if h:
                            nc.scalar.copy(
                                gqb_sb[hp][:, h, q0:q0 + QTW], gqp[:])
                        else:
                            nc.vector.tensor_copy(
                                gqb_sb[hp][:, h, q0:q0 + QTW], gqp[:])
                        nc.scalar.activation(
                            ubc_sb[hp][:, h, q0:q0 + QTW], gqp[:],
                            Act.Exp, scale=-1.0, bias=ebias[:, :])

                    # per-partition gk: gk05 (tanh bias) + vex (recip)
                    for h in range(HPC):
                        hsl = slice(h * DK, (h + 1) * DK)
                        gkp = mis.tile([P, QTW // P], f32, tag="mis",
                                       name="gkp")
                        for j in range(QTW // P):
                            i = qt * (QTW // P) + j
                            nc.tensor.matmul(
                                gkp[:, j:j + 1],
                                lhsT=kt_sb[hp][hsl, i * P:(i + 1) * P],
                                rhs=wgk_sb[hsl, h:h + 1],
                                start=(j == 0), stop=(j == QTW // P - 1),
                                skip_group_check=True)
                        nc.scalar.activation(
                            gk05_sb[hp][:, h, qt * (QTW // P):
                                        (qt + 1) * (QTW // P)],
                            gkp[:], Act.Identity, scale=0.5,
                            bias=gb05[:, :])
                        nc.scalar.activation(
                            vex_sb[hp][:, h, qt * (QTW // P):
                                       (qt + 1) * (QTW // P)],
                            gkp[:], Act.Exp, scale=-1.0)

                # ===== attention chunks whose k-tiles are now ready ====
                ready = [aq for aq in range(NQT)
                         if mc[aq] == qt or (qt == NQT - 1 and mc[aq] > qt)]
                for aq in ready:
                  a0 = aq * QTW
                  otts = {}
                  for hp in range(HPC):
                      probs = {}
                      tiles = []
                      for n_i, i in enumerate(iv_qt[aq]):
                          recip = (n_i % 3 == 1)
                          # first non-skip 128-slice of this (i, aq)
                          sjlo = min(j for j in range(QTW // P)
                                     if st[i, aq * (QTW // P) + j])
                          tiles.append((i, recip, sjlo * P))
                      for (i, recip, off) in tiles:
                          pre = {}
                          for h in range(HPC):
                              if recip:
                                  dn = work.tile([P, QTW], f32,
                                                 tag="den", name="den",
                                                 bufs=4)
                                  nc.gpsimd.tensor_scalar(
                                      dn[:, off:],
                                      ubc_sb[hp][:, h, a0 + off:a0 + QTW],
                                      vex_sb[hp][:, h, i:i + 1], 0.5,
                                      Alu.mult, Alu.add)
                                  pre[(i, h)] = dn
                              else:
                                  tnh = work.tile([P, QTW], b16,
                                                  tag="tnh", name="tnh",
                                                  bufs=6)
                                  nc.scalar.activation(
                                      tnh[:, off:],
                                      gqb_sb[hp][:, h, a0 + off:a0 + QTW],
                                      Act.Tanh, scale=0.5,
                                      bias=gk05_sb[hp][:, h, i:i + 1])
                                  pre[(i, h)] = tnh
                          p3 = prb.tile([P, HPC, QTW], b16, tag=f"pr{i}",
                                        name=f"pr{i}",
                                        bufs=2 if i < 8 else 1)
                          g3 = work.tile([P, HPC, QTW], b16, tag="gat",
                                         name="gat")
                          sjlo = off // P
                          mixed = [sj for sj in range(sjlo, QTW // P)
                                   if st[i, aq * (QTW // P) + sj] == 2]
                          for h in range(HPC):
                              hsl = slice(h * DK, (h + 1) * DK)
                              sch = scp.tile([P, QTW], f32, tag="sc",
                                             name="sch")
                              nc.tensor.matmul(
                                  sch[:, off:],
                                  lhsT=kt_sb[hp][hsl, i * P:(i + 1) * P],
                                  rhs=qt_sb[hp][hsl, a0 + off:a0 + QTW],
                                  start=True, stop=not mixed)
                              # additive mask bias: sc += m01^T . I
                              for n, sj in enumerate(mixed):
                                  s = aq * (QTW // P) + sj
                                  nc.tensor.matmul(
                                      sch[:, sj * P:(sj + 1) * P],
                                      lhsT=m01_sb[:, midx[(i, s)], :],
                                      rhs=id_sb[:],
                                      start=False,
                                      stop=(n == len(mixed) - 1),
                                      skip_group_check=True)
                              if recip:
                                  rc = work.tile([P, QTW], f32, tag="rec",
                                                 name="rec", bufs=3)
                                  nc.vector.reciprocal_approx_fast(
                                      rc[:, off:], pre[(i, h)][:, off:])
                                  nc.vector.tensor_tensor(
                                      g3[:, h, off:], sch[:, off:],
                                      rc[:, off:], Alu.mult)
                              else:
                                  nc.vector.scalar_tensor_tensor(
                                      g3[:, h, off:], pre[(i, h)][:, off:],
                                      1.0, sch[:, off:],
                                      Alu.add, Alu.mult)
                          nc.scalar.activation(
                              p3[:, :, off:], g3[:, :, off:], Act.Exp)
                          probs[i] = p3

                      # attn @ V, normalize, transpose; out-proj deferred
                      ott = work.tile([P, QTW // P, P], b16,
                                      tag=f"ott{hp}", name=f"ott{hp}")
                      otts[hp] = ott
                      for sj in range(QTW // P):
                          s = aq * (QTW // P) + sj
                          onat = work.tile([P, P], b16, tag="onat",
                                           name="onat")
                          ops = att.tile([P, HPC * (DK + 1)], f32, tag="o",
                                         name="ops")
                          vi = valid_i[s]
                          for h in range(HPC):
                              ob = h * (DK + 1)
                              if not vi:
                                  nc.vector.memset(
                                      ops[:, ob:ob + DK + 1], 0.0)
                              for n, i in enumerate(vi):
                                  nc.tensor.matmul(
                                      ops[:, ob:ob + DK + 1],
                                      lhsT=probs[i][:, h,
                                                    sj * P:(sj + 1) * P],
                                      rhs=v2_sb[hp][:, i, h, :],
                                      start=(n == 0),
                                      stop=(n == len(vi) - 1),
                                      skip_group_check=True)
                              recv = work.tile([P, 1], f32, tag="recip",
                                               name="recip", bufs=4)
                              nc.vector.reciprocal_approx_fast(
                                  recv[:], ops[:, ob + DK:ob + DK + 1])
                              nc.vector.tensor_scalar_mul(
                                  onat[:, h * DK:(h + 1) * DK],
                                  ops[:, ob:ob + DK], recv[:])
                          trp = mis.tile([P, P], b16, tag="mis", name="trp")
                          nc.tensor.transpose(trp[:], onat[:], id_sb[:])
                          nc.vector.tensor_copy(ott[:, sj, :], trp[:])
                  # output projection: accumulate both hp per s-slice
                  for sj in range(QTW // P):
                      s = aq * (QTW // P) + sj
                      po = work.tile([P, 2, QTW], b16, tag="po",
                                     name="po", bufs=3)
                      for nt in range(2):
                          pps2 = mis.tile([P, QTW], f32, tag="mis",
                                          name="fps")
                          for hp in range(HPC):
                              nc.tensor.matmul(
                                  pps2[:],
                                  lhsT=otts[hp][:, sj, :],
                                  rhs=wo_sb[:, hp, nt * QTW:(nt + 1) * QTW],
                                  start=(hp == 0), stop=(hp == HPC - 1))
                          nc.vector.tensor_copy(po[:, nt, :], pps2[:])
                      nc.sync.dma_start(
                          outp[s * P:(s + 1) * P, :], po[:])
    nc.compile()
    return nc


def _host_prep(inputs):
    q = np.asarray(inputs["query"], np.float32)
    k = np.asarray(inputs["key"], np.float32)
    v = np.asarray(inputs["value"], np.float32)
    mask = np.asarray(inputs["mask"])
    Wq = np.asarray(inputs["Wq"], np.float32)
    Wk = np.asarray(inputs["Wk"], np.float32)
    Wv = np.asarray(inputs["Wv"], np.float32)
    Wo = np.asarray(inputs["Wo"], np.float32)
    bq = np.asarray(inputs["bq"], np.float32)
    bk = np.asarray(inputs["bk"], np.float32)
    bv = np.asarray(inputs["bv"], np.float32)
    bo = np.asarray(inputs["bo"], np.float32)
    wgq = np.asarray(inputs["wgq"], np.float32)
    wgk = np.asarray(inputs["wgk"], np.float32)
    gb = float(np.asarray(inputs["gb"]))

    st, m01_b, midx = _prep_mask(mask)

    # x in [P, ND, S] batched-DMA layout
    xt_b = [[np.ascontiguousarray(
                x[b].T.reshape(ND, P, S).transpose(1, 0, 2)).astype(bf16)
             for b in range(B)] for x in (q, k, v)]

    def wslice(W, cols, scale=1.0):
        # W.T column slice [D, 128] -> [128, 8, 128] -> [128, 1024]
        wt = (W.T[:, cols:cols + CW] * scale).astype(bf16)
        return np.ascontiguousarray(
            wt.reshape(ND, P, CW).transpose(1, 0, 2).reshape(P, D))

    # q is pre-scaled by 0.5/sqrt(dk): scores arrive as s/2, and the
    # gate multiply computes (tanh(l/2)+1)*(s/2) = sigma(l)*s.
    qscale = 0.5 / np.sqrt(DK)
    ident = np.eye(P, dtype=bf16)

    meta = {
        "st": st, "midx": midx, "n_m01": m01_b[0].shape[1], "gb": gb,
        "use_bq": bool(np.any(bq)), "use_bk": bool(np.any(bk)),
        "use_bv": bool(np.any(bv)),
    }

    # gate weight row layouts: gq lands at rows h*DK of ug2 (compensated
    # for the q pre-scale); gkT lands at rows h*DK+1 of lk2.
    # ebias = -gb - ln2 for the u-exp; gb05 = gb/2 for the tanh bias
    ebias_h = np.full((P, 1), -gb - LN2, np.float32)
    gb05_h = np.full((P, 1), 0.5 * gb, np.float32)

    wgq_bd = np.zeros((P, DKP), np.float32)
    wgk_bd = np.zeros((P, HPC), np.float32)
    for h in range(HPC):
        wgq_bd[h * DK:(h + 1) * DK, h * DK] = wgq / qscale
        wgk_bd[h * DK:(h + 1) * DK, h] = wgk

    ngrp = NCORES // B          # head-groups per batch
    in_maps = []
    for c in range(NCORES):
        bc = c // ngrp          # batch of this core
        hg = c % ngrp           # head-group
        cols = [(hg * HPC + 0) * CW, (hg * HPC + 1) * CW]
        im = {
            "xqt": xt_b[0][bc], "xkt": xt_b[1][bc], "xvt": xt_b[2][bc],
            "wqt": np.ascontiguousarray(np.stack(
                [wslice(Wq, cl, qscale) for cl in cols]).transpose(1, 0, 2)),
            "wkt": np.ascontiguousarray(np.stack(
                [wslice(Wk, cl) for cl in cols]).transpose(1, 0, 2)),
            "wvt": np.ascontiguousarray(np.stack(
                [wslice(Wv, cl) for cl in cols]).transpose(1, 0, 2)),
            "wot": np.ascontiguousarray(np.stack(
                [Wo.T[cl:cl + CW, :].astype(bf16)
                 for cl in cols]).transpose(1, 0, 2)),
            "wgq": wgq_bd.astype(bf16), "wgk": wgk_bd.astype(bf16),
            "ebiasd": ebias_h, "gb05d": gb05_h,
            "identd": ident, "m01d": m01_b[bc],
            "onesd": np.ones((DK + 1, P), bf16),
            "bqt": np.ascontiguousarray(np.stack(
                [(bq[cl:cl + CW] * qscale).reshape(P, 1).astype(np.float32)
                 for cl in cols]).transpose(1, 0, 2)),
            "bkt": np.ascontiguousarray(np.stack(
                [bk[cl:cl + CW].reshape(P, 1).astype(np.float32)
                 for cl in cols]).transpose(1, 0, 2)),
            "bvt": np.ascontiguousarray(np.stack(
                [np.tile(bv[cl:cl + CW], (P, 1)).astype(np.float32)
                 for cl in cols]).transpose(1, 0, 2)),
        }
        in_maps.append(im)
    return meta, in_maps, bo


def kernel(**inputs):
    meta, in_maps, bo = _host_prep(inputs)

    key = (meta["st"].tobytes(), meta["gb"], meta["use_bq"],
           meta["use_bk"], meta["use_bv"], meta["n_m01"])
    if key not in _CACHE:
        _CACHE[key] = _build(meta)
    nc = _CACHE[key]

    from concourse.bass_utils import run_bass_kernel_spmd
    res = run_bass_kernel_spmd(
        nc, in_maps, core_ids=list(range(NCORES)),
        trace=bool(int(os.environ.get("KERNEL_TRACE", "0"))))
    out = np.zeros((B, S, D), np.float32)
    ngrp = NCORES // B
    for c, r in enumerate(res.results):
        out[c // ngrp] += r["outp"].astype(np.float32)
    out += bo
    if res.exec_time_ns is not None:
        print(f"HW exec time: {res.exec_time_ns} ns")
    return out
